# revision 51
# baseline (speedup 1.0000x reference)
"""Al-Salam-Carlitz KAN layer on 8 TRN2 NeuronCores.

Math: y[b,o] = sum_{i,d} P_d(tanh(x[b,i])) * coeffs[i,o,d], where P_d are the
Al-Salam-Carlitz polynomials given by a three-term recurrence in scalars a, q.
Each P_d is a degree-d polynomial in t = tanh(x), so on the host we fold the
(D+1)x(D+1) basis-change matrix into coeffs:

    y[b,o] = bias[o] + sum_{k=1..D} sum_i t[b,i]^k * Cf[i,o,k]

with bias[o] = sum_i Cf[i,o,0] (the k=0 column times t^0 == 1).  This removes
1/8 of the matmul work and leaves the device with: tanh, a bf16 power chain,
and a K=7*1024 contraction done as 448 TensorE matmuls per core.

Sharding: data-parallel over batch (4096 -> 8 x 512).  Each core receives its
x-shard pre-transposed ([I, 512] in bf16, so the contraction dim lands on SBUF
partitions), the folded weights (bf16, pre-laid-out in exact consumption
order for contiguous chunked DMA), and the bias.  No collectives; the host
concatenates the 8 output shards.

Matmul schedule (one core): 8 output tiles yT[oc] = [128 o, 512 b], each
accumulating 56 K-steps in PSUM bank oc.
  Warm-up: ~16 small dummy matmuls on garbage SBUF ramp the PE out of its
    low-power p-state (first ~3-6us run at 1.2 instead of 2.4 GHz) while the
    first x-tile DMA + tanh are still in flight.
  Phase A (j = 0..13): for each j, one matmul into every bank -- consumption
    of power planes is 8x slower than production, so the PE never stalls on
    the tanh/power chain during ramp-up.
  Phase B (oc = 0..7): finish each bank's remaining 42 K-steps back-to-back,
    so banks complete staggered and PSUM evacuation + output DMA overlap the
    next bank's matmuls.  The final bank is accumulated as two 256-column
    halves: half A finishes ~4.5us before the end, hiding its evacuation and
    output DMA entirely; only half B's short tail remains after the last
    matmul.

DMA plan: x tiles 1..7 ride the otherwise-idle GpSimd SWDGE queue right from
the start (per-tile semaphores, so each tanh fires as its tile lands), the
weight stream owns the Sync queue, and outputs go out on the Scalar queue.
This keeps all power planes ready ~10us before Phase B needs them.
"""

import numpy as np
import ml_dtypes

B, I, O, D1 = 4096, 1024, 1024, 8
NCORES = 8
BS = B // NCORES       # batch rows per core (moving free dim of each matmul)
IC = I // 128          # i chunks (contraction tiles per power plane)
OC = O // 128          # o chunks (output partition tiles)
NK = D1 - 1            # power planes k = 1..7
NJ = IC * NK           # K-steps per output tile
NJA = 14               # phase-A K-steps (covers planes of i-chunks 0..1)

# (oc, j) consumption order of the 448 stationary weight tiles
SEQ = [(oc, j) for j in range(NJA) for oc in range(OC)] + \
      [(oc, j) for oc in range(OC) for j in range(NJA, NJ)]
# weight-DMA chunk sizes (tiles): phase A starts fine-grained (the first
# chunk gates the first matmul) then coarsens; phase B is one 42-tile chunk
# per output group, which both minimizes PE semaphore waits and keeps the
# whole group resident.  The first N_GP chunks ride the GpSimd SWDGE queue
# (in parallel with xin0 on the Sync queue) so the PE's ramp is gated only
# by the first tanh, not by the weight stream.
_SIZES = [4, 4, 8, 8, 8, 16, 16, 16, 16, 16] + [NJ - NJA] * OC
N_GP = 0               # leading weight chunks issued from the GpSimd queue
                       # (measured: SWDGE weight transfers arrive ~1.5us
                       # later than the same chunk on the Sync HWDGE queue)
CHUNKS = []
_s = 0
for _sz in _SIZES:
    CHUNKS.append((_s, _sz))
    _s += _sz
assert _s == OC * NJ
_NA = len(_SIZES) - OC                       # number of phase-A chunks

# chunk index whose last matmul completes group oc (phase B: 1 chunk/group)
GROUP_END_CHUNK = [_NA + oc for oc in range(OC)]

_GRAPH = None
LAST_RESULT = None     # BassKernelResults of the most recent run (for test.py)

# split the final output group's accumulation into two 256-col halves so
# half A's evac+store hide under half B's matmuls
SPLIT7 = False

# weight-chunk SBUF ring slots: deep enough that the sync sequencer's
# per-chunk descriptor generation (0.6-3.3us each, run-to-run variable)
# starts early enough for phase-B chunks to land before the PE reaches them.
CW_BUFS = 6

# sync-queue DMA issues hoisted ahead of the framework entry barrier
# (xin0 + the first HOIST_DMAS-1 weight chunks)
HOIST_DMAS = 4


def _build_graph_raw():
    """Raw bacc build: manual per-engine streams + semaphores.  Saves the
    Tile exit drain + double all-engine barrier (~9us) and waits only once
    per weight chunk on the PE instead of per matmul."""
    import concourse.bass as bass
    from concourse import bacc, mybir

    nc = bacc.Bacc("TRN2", target_bir_lowering=False, debug=False,
                   num_devices=NCORES, monotonic_sem_count=0)
    f32 = mybir.dt.float32
    bf16 = mybir.dt.bfloat16

    xT = nc.dram_tensor("xT", [I, BS], bf16, kind="ExternalInput").ap()
    cw = nc.dram_tensor("cw", [128, OC * NJ * 128], bf16,
                        kind="ExternalInput").ap()
    bias = nc.dram_tensor("bias", [128, OC], f32, kind="ExternalInput").ap()
    yT = nc.dram_tensor("yT", [O, BS], f32, kind="ExternalOutput").ap()

    max_chunk = max(sz for _, sz in CHUNKS)
    xin = [nc.alloc_sbuf_tensor(f"xin{i}", [128, BS], bf16).ap()
           for i in range(IC)]
    planes = [nc.alloc_sbuf_tensor(f"pl{j}", [128, BS], bf16).ap()
              for j in range(NJ)]
    cwbuf = [nc.alloc_sbuf_tensor(f"cwb{i}", [128, max_chunk * 128],
                                  bf16).ap()
             for i in range(CW_BUFS)]
    # never written: garbage operand for PE p-state warm-up matmuls
    warm = nc.alloc_sbuf_tensor("warm", [128, 256], bf16).ap()
    warm2 = nc.alloc_sbuf_tensor("warm2", [128, BS], bf16).ap()
    bias_t = nc.alloc_sbuf_tensor("biasb", [128, OC], f32).ap()
    ot = [nc.alloc_sbuf_tensor(f"ot{i}", [128, BS], f32).ap()
          for i in range(2)]
    ps = [nc.alloc_psum_tensor(f"ps{i}", [128, BS], f32).ap()
          for i in range(OC)]
    HB = BS // 2
    ps7a, ps7b = ps[OC - 1][:, 0:HB], ps[OC - 1][:, HB:BS]

    from contextlib import ExitStack
    with ExitStack() as stack:
        # gpsimd issues only early DMAs whose completions are consumed mid-
        # kernel, so its expensive end-of-block dge_drain can be skipped
        block = stack.enter_context(nc.Block(no_gpsimd_drain=True))
        # DMA completion increments land as 16 per-slice +1s, and slices of
        # different in-flight DMAs interleave -- so a semaphore may only be
        # waited at "all DMAs issued on it so far" thresholds.  The weight
        # stream round-robins CW_BUFS semaphores (slot ring ensures only one
        # in-flight DMA per sem); x tiles get one sem each; output slots two.
        cw_dma = [stack.enter_context(nc.semaphore(f"cw_dma{r}"))
                  for r in range(CW_BUFS)]
        # SWDGE and HWDGE may not share a sem: the gpsimd-issued leading
        # chunks get dedicated sems even though they share the buffer ring
        cwg = [stack.enter_context(nc.semaphore(f"cwg{r}"))
               for r in range(N_GP)]
        # xin0 gates the first tanh and rides the sync queue ahead of the
        # weight stream; xins 1..7 go on gpsimd SWDGE with per-tile sems
        # (SWDGE and HWDGE DMAs may not mix on a sem)
        xin0_dma = stack.enter_context(nc.semaphore("xin0_dma"))
        xr_dma = [stack.enter_context(nc.semaphore(f"xr_dma{i}"))
                  for i in range(IC - 1)]
        bias_dma = stack.enter_context(nc.semaphore("bias_dma"))
        bo_dma = stack.enter_context(nc.semaphore("bo_dma"))
        out_dma = [stack.enter_context(nc.semaphore(f"out_dma{r}"))
                   for r in range(2)]
        act_pl = stack.enter_context(nc.semaphore("act_pl"))
        dve_pl = stack.enter_context(nc.semaphore("dve_pl"))
        pe_ch = stack.enter_context(nc.semaphore("pe_ch"))
        pe_half = stack.enter_context(nc.semaphore("pe_half"))
        act_ev = stack.enter_context(nc.semaphore("act_ev"))
        dve_ev = stack.enter_context(nc.semaphore("dve_ev"))

        @block.sync
        def _(eng: bass.BassEngine):
            # xin0 first: it gates the whole plane pipeline
            eng.dma_start(out=xin[0][:], in_=xT[0:128, :]
                          ).then_inc(xin0_dma, 16)
            for ci, (s0, size) in enumerate(CHUNKS):
                if ci < N_GP:
                    continue               # leading chunks ride gpsimd
                if ci >= CW_BUFS:
                    eng.wait_ge(pe_ch, ci - CW_BUFS + 1)
                eng.dma_start(
                    out=cwbuf[ci % CW_BUFS][:, :size * 128],
                    in_=cw[:, s0 * 128:(s0 + size) * 128],
                ).then_inc(cw_dma[ci % CW_BUFS], 16)
            # output stores: the evac->store handoff runs here so the DMA
            # issue cost (~0.6us each) never serializes with the next evac
            # on the Scalar queue
            for oc in range(OC - 1):
                eng.wait_ge(act_ev, oc + 1)
                eng.dma_start(
                    out=yT[oc * 128:(oc + 1) * 128, :],
                    in_=ot[oc % 2][:]
                ).then_inc(out_dma[oc % 2], 16)
            # last group: bias is already in PSUM (K=1 bias matmul), the
            # two halves are copied out by Scalar and Vector in parallel;
            # half A stores from here, half B from the Vector queue
            o0 = (OC - 1) * 128
            eng.wait_ge(act_ev, OC)
            eng.dma_start(out=yT[o0:o0 + 128, 0:HB], in_=ot[1][:, 0:HB]
                          ).then_inc(out_dma[1], 16)

        @block.gpsimd
        def _(eng: bass.BassEngine):
            # leading weight chunks + x tiles 1..7 + bias on the otherwise-
            # idle SWDGE queue, in consumption order
            for ci in range(N_GP):
                s0, size = CHUNKS[ci]
                eng.dma_start(
                    out=cwbuf[ci % CW_BUFS][:, :size * 128],
                    in_=cw[:, s0 * 128:(s0 + size) * 128],
                ).then_inc(cwg[ci], 16)
            for i in range(1, IC):
                eng.dma_start(out=xin[i][:], in_=xT[i * 128:(i + 1) * 128, :]
                              ).then_inc(xr_dma[i - 1], 16)
            eng.dma_start(out=bias_t[:], in_=bias[:]).then_inc(bias_dma, 16)

        @block.scalar
        def _(eng: bass.BassEngine):
            eng.wait_ge(xin0_dma, 16)
            eng.activation(planes[0][:], xin[0][:],
                           mybir.ActivationFunctionType.Tanh
                           ).then_inc(act_pl, 1)
            for i in range(1, IC):
                eng.wait_ge(xr_dma[i - 1], 16)
                eng.activation(planes[i * NK][:], xin[i][:],
                               mybir.ActivationFunctionType.Tanh
                               ).then_inc(act_pl, 1)
            eng.wait_ge(bias_dma, 16)
            for oc in range(OC - 1):
                eng.wait_ge(pe_ch, GROUP_END_CHUNK[oc] + 1)
                if oc >= 2:
                    eng.wait_ge(out_dma[oc % 2], 16 * (oc // 2))
                eng.activation(ot[oc % 2][:], ps[oc][:],
                               mybir.ActivationFunctionType.Identity,
                               bias=bias_t[:, oc:oc + 1]
                               ).then_inc(act_ev, 1)
            # last group, half A: pure copy (bias already accumulated in
            # PSUM); half B is copied by the Vector engine concurrently.
            # No final out-DMA waits -- the runtime drains the queues.
            eng.wait_ge(pe_ch, len(CHUNKS))
            eng.wait_ge(out_dma[1], 16 * ((OC - 1) // 2))
            eng.activation(ot[1][:, 0:HB], ps[OC - 1][:, 0:HB],
                           mybir.ActivationFunctionType.Identity,
                           bias=bias_t[:, OC - 1:OC]).then_inc(act_ev, 1)
            eng.activation(ot[1][:, HB:BS], ps[OC - 1][:, HB:BS],
                           mybir.ActivationFunctionType.Identity,
                           bias=bias_t[:, OC - 1:OC]).then_inc(dve_ev, 1)
            # store half B from here while Sync issues half A
            o0 = (OC - 1) * 128
            eng.wait_ge(dve_ev, 1)
            eng.dma_start(out=yT[o0:o0 + 128, HB:BS],
                          in_=ot[1][:, HB:BS]).then_inc(out_dma[1], 16)

        @block.vector
        def _(eng: bass.BassEngine):
            for i in range(IC):
                eng.wait_ge(act_pl, i + 1)
                for k1 in range(1, NK):
                    if k1 >= 2:
                        # same-engine RAW still needs a sem wait (deep
                        # pipeline, no interlock)
                        eng.wait_ge(dve_pl, i * (NK - 1) + k1 - 1)
                    eng.tensor_mul(planes[i * NK + k1][:],
                                   planes[i * NK + k1 - 1][:],
                                   planes[i * NK][:]
                                   ).then_inc(dve_pl, 1)


        @block.tensor
        def _(eng: bass.BassEngine):
            # p-state warm-up on garbage inputs: no waits, runs while the
            # first x tile + weight chunk DMAs are in flight, so the real
            # stream starts at (or near) full clock
            for _ in range(5):
                eng.matmul(ps[0][:], warm2[:, 0:128], warm2[:],
                           start=True, stop=True)
            doneA = [0] * OC
            doneB = 0
            seen_act = seen_dve = 0
            sem_uses = [0] * CW_BUFS   # HWDGE waits per ring sem
            for ci, (s0, size) in enumerate(CHUNKS):
                # attach all of the chunk's waits to its first matmul --
                # the move_matmul_waits_to_ldweights compile pass hoists
                # them onto the LDWEIGHTS, keeping the PE's 64-deep
                # reorder window free to pull later weight loads ahead
                js = [SEQ[s][1] for s in range(s0, s0 + size)]
                need_act = max((j // NK + 1 for j in js if j % NK == 0),
                               default=0)
                need_dve = max((j // NK * (NK - 1) + j % NK
                                for j in js if j % NK != 0), default=0)
                if need_act > seen_act:
                    eng.wait_ge(act_pl, need_act)
                    seen_act = need_act
                if need_dve > seen_dve:
                    eng.wait_ge(dve_pl, need_dve)
                    seen_dve = need_dve
                last7 = (ci == len(CHUNKS) - 1) and SPLIT7   # two sweeps
                for t in range(size):
                    oc, j = SEQ[s0 + t]
                    cwap = cwbuf[ci % CW_BUFS][:, t * 128:(t + 1) * 128]
                    if oc < OC - 1 or not SPLIT7:
                        mm = eng.matmul(ps[oc][:], cwap, planes[j][:],
                                        start=(doneA[oc] == 0),
                                        stop=(doneA[oc] == NJ - 1))
                        doneA[oc] += 1
                    else:
                        mm = eng.matmul(ps7a, cwap, planes[j][:, 0:HB],
                                        start=(doneA[oc] == 0),
                                        stop=(doneA[oc] == NJ - 1))
                        if doneA[oc] == NJ - 1:
                            mm.then_inc(pe_half, 1)
                        doneA[oc] += 1
                        if not last7:
                            # phase A: same stationary tile, second half
                            eng.matmul(ps7b, cwap, planes[j][:, HB:BS],
                                       start=(doneB == 0), stop=False)
                            doneB += 1
                    if t == 0:
                        if ci < N_GP:
                            mm._wait_ge(cwg[ci], 16)
                        else:
                            sem_uses[ci % CW_BUFS] += 1
                            mm._wait_ge(cw_dma[ci % CW_BUFS],
                                        16 * sem_uses[ci % CW_BUFS])
                    if t == size - 1 and not last7:
                        mm.then_inc(pe_ch, 1)
                if last7:
                    # second sweep: half B of the final group (re-loads the
                    # 42 stationary tiles; LDWEIGHTS hides under the 256-col
                    # matmuls)
                    for t in range(size):
                        _, j = SEQ[s0 + t]
                        mm = eng.matmul(
                            ps7b,
                            cwbuf[ci % CW_BUFS][:, t * 128:(t + 1) * 128],
                            planes[j][:, HB:BS],
                            start=(doneB == 0), stop=(doneB == NJ - 1))
                        doneB += 1
                        if t == size - 1:
                            mm.then_inc(pe_ch, 1)
            assert all(d == NJ for d in doneA)
            assert doneB == (NJ if SPLIT7 else 0)

    # Hoist the first few Sync-queue DMA issues (xin0 + leading weight
    # chunks) into the entry block, ahead of the framework's all-engine
    # barrier: the ~0.6us-per-DMA descriptor generation then overlaps the
    # fixed ~7us NEFF preamble instead of following it.  Safe because these
    # DMAs write SBUF regions nothing reads until their semaphores fire,
    # and the semaphores start at zero regardless of the barrier.
    from concourse import mybir as _mybir
    entry = nc.main_func.blocks[0]
    sp_eng = _mybir.EngineType.SP
    sp_body = next(
        b for b in nc.main_func.blocks
        if b.instructions and type(b.instructions[0]).__name__ == "InstDMACopy"
        and b.instructions[0].engine == sp_eng)
    moved = []
    for inst in list(sp_body.instructions):
        if len(moved) >= HOIST_DMAS:
            break
        if type(inst).__name__ != "InstDMACopy":
            break
        moved.append(inst)
    # place them AFTER SP's barrier-arrival drain (so the other engines'
    # release isn't delayed by the DMA issues) but BEFORE its release-wait
    bar_idx = next(
        i for i, inst in enumerate(entry.instructions)
        if type(inst).__name__ == "InstDrain" and inst.engine == sp_eng)
    for inst in moved:
        sp_body.instructions.remove(inst)
    for k, inst in enumerate(moved):
        entry.instructions.insert(bar_idx + 1 + k, inst)

    nc.compile()
    return nc


def _get_graph():
    global _GRAPH
    if _GRAPH is None:
        _GRAPH = _build_graph_raw()
    return _GRAPH


def _host_prep(a, q, coeffs):
    """Fold the polynomial basis change into the weights (float64 on host)."""
    # c[d, k]: P_d(t) = sum_k c[d, k] * t^k, from the three-term recurrence
    c = np.zeros((D1, D1), np.float64)
    c[0, 0] = 1.0
    if D1 > 1:
        c[1, 1] = 1.0
        c[1, 0] = -a
    for n in range(2, D1):
        c[n, 1:] += c[n - 1, :-1]
        c[n, :] -= (a + q ** n) * c[n - 1, :]
        c[n, :] -= a * q ** (n - 1) * c[n - 2, :]

    Cf = (coeffs.reshape(-1, D1).astype(np.float64) @ c).reshape(I, O, D1)
    bias = Cf[:, :, 0].sum(axis=0).astype(np.float32)                # [O]
    Ck = Cf[:, :, 1:].astype(np.float32).astype(ml_dtypes.bfloat16)  # [I,O,NK]

    # stationary tile for (oc, j=ic*NK+k1): [128 i-part, 128 o-col] slice
    t = Ck.reshape(IC, 128, OC, 128, NK)            # [ic, p, oc, ol, k1]
    X = np.ascontiguousarray(t.transpose(2, 0, 4, 1, 3)) \
          .reshape(OC, NJ, 128, 128)                # [oc, j, p, ol]
    oc_idx = np.array([oc for oc, _ in SEQ])
    j_idx = np.array([j for _, j in SEQ])
    seq_tiles = X[oc_idx, j_idx]                    # [448, p, ol]
    cw_dev = np.ascontiguousarray(
        seq_tiles.transpose(1, 0, 2)).reshape(128, OC * NJ * 128)
    bias_dev = np.ascontiguousarray(bias.reshape(OC, 128).T)  # [128, OC]
    return cw_dev, bias_dev


def _ensure_axon_hooks_importable():
    """run_bass_kernel_spmd imports antenv.axon_hooks when BASS_TRACE is
    set; some images lack that module.  Register a no-op fallback so a
    trace request degrades to a warning instead of an ImportError."""
    import sys
    import types
    if "antenv.axon_hooks" in sys.modules:
        return
    try:
        import antenv.axon_hooks  # noqa: F401
    except ImportError:
        mod = types.ModuleType("antenv.axon_hooks")
        state = {"hook": None}
        mod.set_axon_ntff_profile_hook = \
            lambda h: state.__setitem__("hook", h)
        mod.get_axon_ntff_profile_hook = lambda: state["hook"]
        sys.modules["antenv.axon_hooks"] = mod
        try:
            import antenv
            antenv.axon_hooks = mod
        except ImportError:
            pass


def kernel(x, a, q, coeffs):
    global LAST_RESULT
    _ensure_axon_hooks_importable()
    from concourse.bass_utils import run_bass_kernel_spmd

    x = np.ascontiguousarray(np.asarray(x, dtype=np.float32))
    coeffs = np.ascontiguousarray(np.asarray(coeffs, dtype=np.float32))
    a_val = float(np.asarray(a).reshape(-1)[0])
    q_val = float(np.asarray(q).reshape(-1)[0])

    cw_dev, bias_dev = _host_prep(a_val, q_val, coeffs)
    xs = x.reshape(NCORES, BS, I).transpose(0, 2, 1)  # [core, I, BS]
    xs = xs.astype(ml_dtypes.bfloat16)

    in_maps = [{
        "xT": np.ascontiguousarray(xs[c]),
        "cw": cw_dev,
        "bias": bias_dev,
    } for c in range(NCORES)]

    nc = _get_graph()
    res = run_bass_kernel_spmd(nc, in_maps, core_ids=list(range(NCORES)))
    LAST_RESULT = res

    shards = [np.asarray(res.results[c]["yT"]).T for c in range(NCORES)]
    return np.ascontiguousarray(np.concatenate(shards, axis=0),
                                dtype=np.float32)


if __name__ == "__main__":
    rng = np.random.default_rng(0)
    inputs = {
        "x": rng.standard_normal((B, I), dtype=np.float32),
        "a": np.zeros((1,), np.float32),
        "q": np.ones((1,), np.float32),
        "coeffs": rng.standard_normal((I, O, D1), dtype=np.float32)
        / (I * D1),
    }
    y = kernel(**inputs)
    print("out", y.shape, y.dtype, float(np.abs(y).mean()))


# revision 52
# speedup vs baseline: 1.2740x; 1.2740x over previous
"""Al-Salam-Carlitz KAN layer on 8 TRN2 NeuronCores.

Math: y[b,o] = sum_{i,d} P_d(tanh(x[b,i])) * coeffs[i,o,d], where P_d are the
Al-Salam-Carlitz polynomials given by a three-term recurrence in scalars a, q.
Each P_d is a degree-d polynomial in t = tanh(x), so on the host we fold the
(D+1)x(D+1) basis-change matrix into coeffs:

    y[b,o] = bias[o] + sum_{k=1..7} sum_i t[b,i]^k * Cf[i,o,k]

with bias[o] = sum_i Cf[i,o,0] (the k=0 column times t^0 == 1).

Mixed precision: after basis folding the per-k weight norms are wildly
uneven -- k=3,4,5 carry ~89% of the output variance, k=1,2,6,7 only ~11%.
The low-variance planes run as fp8-e4m3 DoubleRow matmuls (2 K-tiles per
instruction, measured 2x bf16 throughput at 512 moving cols), the heavy
k=3,4,5 stay bf16.  Per output group: 8 i-chunks x (1 pair(k1,k2) +
3 bf16 singles + 1 pair(k6,k7)) = 40 matmul steps instead of 56.
Measured end-to-end rel err ~1e-2 vs the 2e-2 gate.

fp8 weight encoding needs a scale: the folded weights (sigma ~1e-4..2e-3)
sit below e4m3's subnormal floor, so ALL weights are pre-scaled by 2^13 on
the host and the PSUM is descaled in the evacuation (activation
out = in*scale + bias with scale = 2^-13, an exact power of two).

Sharding: data-parallel over batch (4096 -> 8 x 512).  Each core receives
its x-shard pre-transposed ([I, 512] bf16), the folded weight stream (one
fp8-typed byte stream; bf16 tiles are bitcast views, every step is 256
bytes/partition), and the bias.  No collectives.

Schedule (one core): 8 PSUM banks, each accumulating its 40 steps.
  Entry: the first 4 Sync DMA issues (xin0 + 3 weight chunks) are hoisted
    into the NEFF entry block ahead of the framework's all-engine barrier,
    so their descriptors generate during the fixed ~7us preamble.
  Warm-up: 5 dummy matmuls on garbage ramp the PE p-state while the first
    tanh/fp8-pair is still in flight.
  Phase A (steps 0..19 = i-chunks 0..3): one step per bank round-robin, so
    plane production (ACT tanh + fp8 copies, DVE power chain) stays ahead.
  Phase B (per bank, steps 20..39): back-to-back finish, staggered bank
    completion; evac + store overlap the next bank's matmuls.  Final group
    is evacuated in two column halves with the stores issued from the Sync
    and Scalar queues in parallel.
"""

import numpy as np
import ml_dtypes

B, I, O, D1 = 4096, 1024, 1024, 8
NCORES = 8
BS = B // NCORES       # batch rows per core (moving free dim of each matmul)
IC = I // 128          # i chunks
OC = O // 128          # o chunks (output partition tiles / PSUM banks)
SPI = 5                # steps per i-chunk: [pair(k1,k2), k3, k4, k5, pair(k6,k7)]
NJ_S = IC * SPI        # 40 steps per output group
NJA_S = 20             # phase-A steps (covers i-chunks 0..3)
STEP_B = 256           # weight-stream bytes per partition per step
NSTEP = OC * NJ_S      # 320 total steps
WSCALE = 8192.0        # 2^13 weight pre-scale (fp8 dynamic range)
FP8_MAX = 240.0        # TRN e4m3 saturates at +-240 (not OCP's 448)

# (oc, j) consumption order of the 320 weight-stream steps
SEQ = [(oc, j) for j in range(NJA_S) for oc in range(OC)] + \
      [(oc, j) for oc in range(OC) for j in range(NJA_S, NJ_S)]
# weight-DMA chunk sizes (steps): phase A starts fine-grained (the first
# chunk gates the first matmul) then coarsens; phase B is one chunk per
# output group
_SIZES = [4, 4, 8, 8, 8, 16, 16, 24, 24, 24, 24] + [NJ_S - NJA_S] * OC
CHUNKS = []
_s = 0
for _sz in _SIZES:
    CHUNKS.append((_s, _sz))
    _s += _sz
assert _s == NSTEP
_NA = len(_SIZES) - OC                       # number of phase-A chunks
GROUP_END_CHUNK = [_NA + oc for oc in range(OC)]

_GRAPH = None
LAST_RESULT = None     # BassKernelResults of the most recent run (for test.py)

# weight-chunk SBUF ring slots
CW_BUFS = 6
# sync-queue DMA issues hoisted ahead of the framework entry barrier
# (xin0 + the first HOIST_DMAS-1 weight chunks)
HOIST_DMAS = 4


def _step_meta(j):
    """kind ('p12'|'bf'|'p67'), bf16 power k, act/dve sem thresholds."""
    ic, s = divmod(j, SPI)
    if s == 0:
        return ic, 'p12', None, 3 * ic + 3, 0
    if s == 4:
        return ic, 'p67', None, 0, 6 * ic + 6
    return ic, 'bf', s + 2, 0, 6 * ic + s + 1   # s=1,2,3 -> k=3,4,5


def _build_graph_raw():
    import concourse.bass as bass
    from concourse import bacc, mybir

    nc = bacc.Bacc("TRN2", target_bir_lowering=False, debug=False,
                   num_devices=NCORES, monotonic_sem_count=0)
    f32 = mybir.dt.float32
    bf16 = mybir.dt.bfloat16
    fp8 = mybir.dt.float8e4

    xT = nc.dram_tensor("xT", [I, BS], bf16, kind="ExternalInput").ap()
    cw = nc.dram_tensor("cw", [128, NSTEP * STEP_B], fp8,
                        kind="ExternalInput").ap()
    bias = nc.dram_tensor("bias", [128, OC], f32, kind="ExternalInput").ap()
    yT = nc.dram_tensor("yT", [O, BS], f32, kind="ExternalOutput").ap()

    max_chunk = max(sz for _, sz in CHUNKS)
    xin = [nc.alloc_sbuf_tensor(f"xin{i}", [128, BS], bf16).ap()
           for i in range(IC)]
    # bf16 planes per i-chunk: index by k (1..5); k=1 is tanh
    pbf = [{k: nc.alloc_sbuf_tensor(f"pb{i}_{k}", [128, BS], bf16).ap()
            for k in range(1, 6)} for i in range(IC)]
    pr12 = [nc.alloc_sbuf_tensor(f"p12_{i}", [128, 2, BS], fp8).ap()
            for i in range(IC)]
    pr67 = [nc.alloc_sbuf_tensor(f"p67_{i}", [128, 2, BS], fp8).ap()
            for i in range(IC)]
    cwbuf = [nc.alloc_sbuf_tensor(f"cwb{i}", [128, max_chunk * STEP_B],
                                  fp8).ap()
             for i in range(CW_BUFS)]
    # never written: garbage operand for PE p-state warm-up matmuls
    warm2 = nc.alloc_sbuf_tensor("warm2", [128, BS], bf16).ap()
    bias_t = nc.alloc_sbuf_tensor("biasb", [128, OC], f32).ap()
    ot = [nc.alloc_sbuf_tensor(f"ot{i}", [128, BS], f32).ap()
          for i in range(2)]
    ps = [nc.alloc_psum_tensor(f"ps{i}", [128, BS], f32).ap()
          for i in range(OC)]
    HB = BS // 2

    from contextlib import ExitStack
    with ExitStack() as stack:
        # gpsimd issues only early DMAs whose completions are consumed mid-
        # kernel, so its expensive end-of-block dge_drain can be skipped
        block = stack.enter_context(nc.Block(no_gpsimd_drain=True))
        # DMA completion increments land as 16 per-slice +1s, and slices of
        # different in-flight DMAs interleave -- a semaphore may only be
        # waited at "all DMAs issued on it so far" thresholds.
        cw_dma = [stack.enter_context(nc.semaphore(f"cw_dma{r}"))
                  for r in range(CW_BUFS)]
        xin0_dma = stack.enter_context(nc.semaphore("xin0_dma"))
        # x tiles 1..7 ride gpsimd SWDGE with per-tile sems (SWDGE and
        # HWDGE DMAs may not mix on a sem)
        xr_dma = [stack.enter_context(nc.semaphore(f"xr_dma{i}"))
                  for i in range(IC - 1)]
        bias_dma = stack.enter_context(nc.semaphore("bias_dma"))
        out_dma = [stack.enter_context(nc.semaphore(f"out_dma{r}"))
                   for r in range(2)]
        act_pl = stack.enter_context(nc.semaphore("act_pl"))
        dve_pl = stack.enter_context(nc.semaphore("dve_pl"))
        pe_ch = stack.enter_context(nc.semaphore("pe_ch"))
        act_ev = stack.enter_context(nc.semaphore("act_ev"))
        dve_ev = stack.enter_context(nc.semaphore("dve_ev"))

        @block.sync
        def _(eng: bass.BassEngine):
            # xin0 first: it gates the whole plane pipeline.  This DMA and
            # the first weight chunks are hoisted pre-barrier below.
            eng.dma_start(out=xin[0][:], in_=xT[0:128, :]
                          ).then_inc(xin0_dma, 16)
            for ci, (s0, size) in enumerate(CHUNKS):
                if ci >= CW_BUFS:
                    eng.wait_ge(pe_ch, ci - CW_BUFS + 1)
                eng.dma_start(
                    out=cwbuf[ci % CW_BUFS][:, :size * STEP_B],
                    in_=cw[:, s0 * STEP_B:(s0 + size) * STEP_B],
                ).then_inc(cw_dma[ci % CW_BUFS], 16)
            # output stores: evac->store handoff runs here so the DMA issue
            # cost never serializes with the next evac on the Scalar queue
            for oc in range(OC - 1):
                eng.wait_ge(act_ev, oc + 1)
                eng.dma_start(
                    out=yT[oc * 128:(oc + 1) * 128, :],
                    in_=ot[oc % 2][:]
                ).then_inc(out_dma[oc % 2], 16)
            o0 = (OC - 1) * 128
            eng.wait_ge(act_ev, OC)
            eng.dma_start(out=yT[o0:o0 + 128, 0:HB], in_=ot[1][:, 0:HB]
                          ).then_inc(out_dma[1], 16)

        @block.gpsimd
        def _(eng: bass.BassEngine):
            # x tiles 1..7 + bias on the otherwise-idle SWDGE queue
            for i in range(1, IC):
                eng.dma_start(out=xin[i][:], in_=xT[i * 128:(i + 1) * 128, :]
                              ).then_inc(xr_dma[i - 1], 16)
            eng.dma_start(out=bias_t[:], in_=bias[:]).then_inc(bias_dma, 16)

        @block.scalar
        def _(eng: bass.BassEngine):
            # plane production: tanh (bf16 chain input) + the two fp8
            # copies of the (k1,k2) DoubleRow pair.  act_pl: 3 per i-chunk.
            for i in range(IC):
                if i == 0:
                    eng.wait_ge(xin0_dma, 16)
                else:
                    eng.wait_ge(xr_dma[i - 1], 16)
                eng.activation(pbf[i][1][:], xin[i][:],
                               mybir.ActivationFunctionType.Tanh
                               ).then_inc(act_pl, 1)
                eng.activation(pr12[i][:, 0], pbf[i][1][:],
                               mybir.ActivationFunctionType.Copy
                               ).then_inc(act_pl, 1)
                eng.wait_ge(dve_pl, 6 * i + 1)
                eng.activation(pr12[i][:, 1], pbf[i][2][:],
                               mybir.ActivationFunctionType.Copy
                               ).then_inc(act_pl, 1)
            eng.wait_ge(bias_dma, 16)
            for oc in range(OC - 1):
                eng.wait_ge(pe_ch, GROUP_END_CHUNK[oc] + 1)
                if oc >= 2:
                    eng.wait_ge(out_dma[oc % 2], 16 * (oc // 2))
                eng.activation(ot[oc % 2][:], ps[oc][:],
                               mybir.ActivationFunctionType.Identity,
                               bias=bias_t[:, oc:oc + 1],
                               scale=1.0 / WSCALE).then_inc(act_ev, 1)
            # last group: two half-column evacs; half A stores from Sync,
            # half B from here (Sync is busy issuing half A then)
            eng.wait_ge(pe_ch, len(CHUNKS))
            eng.wait_ge(out_dma[1], 16 * ((OC - 1) // 2))
            eng.activation(ot[1][:, 0:HB], ps[OC - 1][:, 0:HB],
                           mybir.ActivationFunctionType.Identity,
                           bias=bias_t[:, OC - 1:OC],
                           scale=1.0 / WSCALE).then_inc(act_ev, 1)
            eng.activation(ot[1][:, HB:BS], ps[OC - 1][:, HB:BS],
                           mybir.ActivationFunctionType.Identity,
                           bias=bias_t[:, OC - 1:OC],
                           scale=1.0 / WSCALE).then_inc(dve_ev, 1)
            o0 = (OC - 1) * 128
            eng.wait_ge(dve_ev, 1)
            eng.dma_start(out=yT[o0:o0 + 128, HB:BS],
                          in_=ot[1][:, HB:BS]).then_inc(out_dma[1], 16)
            # no final out-DMA waits: the runtime drains the queues

        @block.vector
        def _(eng: bass.BassEngine):
            # power chain t^2..t^5 in bf16 + the (k6,k7) fp8 pair.
            # dve_pl: 6 per i-chunk.  Same-engine RAW still needs a sem wait
            # (deep pipeline, no interlock).
            for i in range(IC):
                t = pbf[i][1]
                eng.wait_ge(act_pl, 3 * i + 1)
                eng.tensor_mul(pbf[i][2][:], t[:], t[:]).then_inc(dve_pl, 1)
                eng.wait_ge(dve_pl, 6 * i + 1)
                eng.tensor_mul(pbf[i][3][:], pbf[i][2][:], t[:]
                               ).then_inc(dve_pl, 1)
                eng.wait_ge(dve_pl, 6 * i + 2)
                eng.tensor_mul(pbf[i][4][:], pbf[i][3][:], t[:]
                               ).then_inc(dve_pl, 1)
                eng.wait_ge(dve_pl, 6 * i + 3)
                eng.tensor_mul(pbf[i][5][:], pbf[i][4][:], t[:]
                               ).then_inc(dve_pl, 1)
                eng.wait_ge(dve_pl, 6 * i + 4)
                eng.tensor_mul(pr67[i][:, 0], pbf[i][5][:], t[:]
                               ).then_inc(dve_pl, 1)
                eng.tensor_mul(pr67[i][:, 1], pbf[i][5][:], pbf[i][2][:]
                               ).then_inc(dve_pl, 1)

        @block.tensor
        def _(eng: bass.BassEngine):
            # p-state warm-up on garbage inputs while the first x tile +
            # weight chunk DMAs land
            for _ in range(5):
                eng.matmul(ps[0][:], warm2[:, 0:128], warm2[:],
                           start=True, stop=True)
            done = [0] * OC
            seen_act = seen_dve = 0
            sem_uses = [0] * CW_BUFS
            for ci, (s0, size) in enumerate(CHUNKS):
                js = [SEQ[s][1] for s in range(s0, s0 + size)]
                need_act = max(_step_meta(j)[3] for j in js)
                need_dve = max(_step_meta(j)[4] for j in js)
                if need_act > seen_act:
                    eng.wait_ge(act_pl, need_act)
                    seen_act = need_act
                if need_dve > seen_dve:
                    eng.wait_ge(dve_pl, need_dve)
                    seen_dve = need_dve
                for t in range(size):
                    oc, j = SEQ[s0 + t]
                    ic, kind, kk, _, _ = _step_meta(j)
                    sl = cwbuf[ci % CW_BUFS][:,
                                             t * STEP_B:(t + 1) * STEP_B]
                    if kind == 'bf':
                        mm = eng.matmul(ps[oc][:], sl.bitcast(bf16),
                                        pbf[ic][kk][:],
                                        start=(done[oc] == 0),
                                        stop=(done[oc] == NJ_S - 1))
                    else:
                        pair = pr12[ic] if kind == 'p12' else pr67[ic]
                        mm = eng.matmul(
                            ps[oc][:],
                            sl.rearrange("p (two f) -> p two f", two=2),
                            pair[:],
                            start=(done[oc] == 0),
                            stop=(done[oc] == NJ_S - 1),
                            perf_mode=mybir.MatmulPerfMode.DoubleRow)
                    done[oc] += 1
                    if t == 0:
                        sem_uses[ci % CW_BUFS] += 1
                        mm._wait_ge(cw_dma[ci % CW_BUFS],
                                    16 * sem_uses[ci % CW_BUFS])
                    if t == size - 1:
                        mm.then_inc(pe_ch, 1)
            assert all(d == NJ_S for d in done)

    # Hoist the first few Sync-queue DMA issues (xin0 + leading weight
    # chunks) into the entry block, ahead of the framework's all-engine
    # barrier: their ~0.6us-per-DMA descriptor generation then overlaps the
    # fixed ~7us NEFF preamble.  Safe: these DMAs write SBUF regions nothing
    # reads until their semaphores fire, and sems start at zero.
    from concourse import mybir as _mybir
    entry = nc.main_func.blocks[0]
    sp_eng = _mybir.EngineType.SP
    sp_body = next(
        b for b in nc.main_func.blocks
        if b.instructions and type(b.instructions[0]).__name__ == "InstDMACopy"
        and b.instructions[0].engine == sp_eng)
    moved = []
    for inst in list(sp_body.instructions):
        if len(moved) >= HOIST_DMAS:
            break
        if type(inst).__name__ != "InstDMACopy":
            break
        moved.append(inst)
    bar_idx = next(
        i for i, inst in enumerate(entry.instructions)
        if type(inst).__name__ == "InstDrain" and inst.engine == sp_eng)
    for inst in moved:
        sp_body.instructions.remove(inst)
    for k, inst in enumerate(moved):
        entry.instructions.insert(bar_idx + k, inst)

    nc.compile()
    return nc


def _get_graph():
    global _GRAPH
    if _GRAPH is None:
        _GRAPH = _build_graph_raw()
    return _GRAPH


def _host_prep(a, q, coeffs):
    """Fold the polynomial basis change into the weights (float64 on host)
    and pack the mixed bf16/fp8 weight stream."""
    f8 = ml_dtypes.float8_e4m3fn
    bf = ml_dtypes.bfloat16
    # c[d, k]: P_d(t) = sum_k c[d, k] * t^k, from the three-term recurrence
    c = np.zeros((D1, D1), np.float64)
    c[0, 0] = 1.0
    if D1 > 1:
        c[1, 1] = 1.0
        c[1, 0] = -a
    for n in range(2, D1):
        c[n, 1:] += c[n - 1, :-1]
        c[n, :] -= (a + q ** n) * c[n - 1, :]
        c[n, :] -= a * q ** (n - 1) * c[n - 2, :]

    Cf = (coeffs.reshape(-1, D1).astype(np.float64) @ c).reshape(I, O, D1)
    bias_dev = np.ascontiguousarray(
        Cf[:, :, 0].sum(axis=0).astype(np.float32).reshape(OC, 128).T)

    W = Cf[:, :, 1:] * WSCALE                     # [I, O, 7], k index 0..6
    # per-(ic, oc) 128x128 tiles, k = 1..7
    Wt = W.reshape(IC, 128, OC, 128, 7)           # [ic, p, oc, ol, k-1]

    def tile(ic, oc, k):
        return Wt[ic, :, oc, :, k - 1]            # [128, 128] float64

    stream = np.zeros((128, NSTEP * STEP_B), np.uint8)
    for n, (oc, j) in enumerate(SEQ):
        ic, s = divmod(j, SPI)
        dst = stream[:, n * STEP_B:(n + 1) * STEP_B]
        if s in (1, 2, 3):
            tb = tile(ic, oc, s + 2).astype(np.float32).astype(bf)
            dst[:] = tb.view(np.uint8).reshape(128, STEP_B)
        else:
            ka, kb = (1, 2) if s == 0 else (6, 7)
            pa = np.clip(tile(ic, oc, ka), -FP8_MAX, FP8_MAX
                         ).astype(np.float32).astype(f8)
            pb = np.clip(tile(ic, oc, kb), -FP8_MAX, FP8_MAX
                         ).astype(np.float32).astype(f8)
            dst[:, 0:128] = pa.view(np.uint8)
            dst[:, 128:256] = pb.view(np.uint8)
    cw_dev = stream.view(f8)
    return cw_dev, bias_dev


def _ensure_axon_hooks_importable():
    """run_bass_kernel_spmd imports antenv.axon_hooks when BASS_TRACE is
    set; some images lack that module.  Register a no-op fallback so a
    trace request degrades to a warning instead of an ImportError."""
    import sys
    import types
    if "antenv.axon_hooks" in sys.modules:
        return
    try:
        import antenv.axon_hooks  # noqa: F401
    except ImportError:
        mod = types.ModuleType("antenv.axon_hooks")
        state = {"hook": None}
        mod.set_axon_ntff_profile_hook = \
            lambda h: state.__setitem__("hook", h)
        mod.get_axon_ntff_profile_hook = lambda: state["hook"]
        sys.modules["antenv.axon_hooks"] = mod
        try:
            import antenv
            antenv.axon_hooks = mod
        except ImportError:
            pass


def kernel(x, a, q, coeffs):
    global LAST_RESULT
    _ensure_axon_hooks_importable()
    from concourse.bass_utils import run_bass_kernel_spmd

    x = np.ascontiguousarray(np.asarray(x, dtype=np.float32))
    coeffs = np.ascontiguousarray(np.asarray(coeffs, dtype=np.float32))
    a_val = float(np.asarray(a).reshape(-1)[0])
    q_val = float(np.asarray(q).reshape(-1)[0])

    cw_dev, bias_dev = _host_prep(a_val, q_val, coeffs)
    xs = x.reshape(NCORES, BS, I).transpose(0, 2, 1)  # [core, I, BS]
    xs = xs.astype(ml_dtypes.bfloat16)

    in_maps = [{
        "xT": np.ascontiguousarray(xs[c]),
        "cw": cw_dev,
        "bias": bias_dev,
    } for c in range(NCORES)]

    nc = _get_graph()
    res = run_bass_kernel_spmd(nc, in_maps, core_ids=list(range(NCORES)))
    LAST_RESULT = res

    shards = [np.asarray(res.results[c]["yT"]).T for c in range(NCORES)]
    return np.ascontiguousarray(np.concatenate(shards, axis=0),
                                dtype=np.float32)


if __name__ == "__main__":
    rng = np.random.default_rng(0)
    inputs = {
        "x": rng.standard_normal((B, I), dtype=np.float32),
        "a": np.zeros((1,), np.float32),
        "q": np.ones((1,), np.float32),
        "coeffs": rng.standard_normal((I, O, D1), dtype=np.float32)
        / (I * D1),
    }
    y = kernel(**inputs)
    print("out", y.shape, y.dtype, float(np.abs(y).mean()))


# revision 54
# speedup vs baseline: 1.3114x; 1.0293x over previous
"""Al-Salam-Carlitz KAN layer on 8 TRN2 NeuronCores.

Math: y[b,o] = sum_{i,d} P_d(tanh(x[b,i])) * coeffs[i,o,d], where P_d are the
Al-Salam-Carlitz polynomials given by a three-term recurrence in scalars a, q.
Each P_d is a degree-d polynomial in t = tanh(x), so on the host we fold the
(D+1)x(D+1) basis-change matrix into coeffs:

    y[b,o] = bias[o] + sum_{k=1..7} sum_i t[b,i]^k * Cf[i,o,k]

with bias[o] = sum_i Cf[i,o,0] (the k=0 column times t^0 == 1).

Mixed precision: after basis folding the per-k weight norms are wildly
uneven -- k=3,4,5 carry ~89% of the output variance, k=1,2,6,7 only ~11%.
The low-variance planes run as fp8-e4m3 DoubleRow matmuls (2 K-tiles per
instruction, measured 2x bf16 throughput at 512 moving cols), the heavy
k=3,4,5 stay bf16.  Per output group: 8 i-chunks x (1 pair(k1,k2) +
3 bf16 singles + 1 pair(k6,k7)) = 40 matmul steps instead of 56.
Measured end-to-end rel err ~1e-2 vs the 2e-2 gate.

fp8 weight encoding needs a scale: the folded weights (sigma ~1e-4..2e-3)
sit below e4m3's subnormal floor, so ALL weights are pre-scaled by 2^13 on
the host and the PSUM is descaled in the evacuation (activation
out = in*scale + bias with scale = 2^-13, an exact power of two).

Sharding: data-parallel over batch (4096 -> 8 x 512).  Each core receives
its x-shard pre-transposed ([I, 512] bf16), the folded weight stream (one
fp8-typed byte stream; bf16 tiles are bitcast views, every step is 256
bytes/partition), and the bias.  No collectives.

Schedule (one core): 8 PSUM banks, each accumulating its 40 steps.
  Entry: the first 4 Sync DMA issues (xin0 + 3 weight chunks) are hoisted
    into the NEFF entry block ahead of the framework's all-engine barrier,
    so their descriptors generate during the fixed ~7us preamble.
  Warm-up: 5 dummy matmuls on garbage ramp the PE p-state while the first
    tanh/fp8-pair is still in flight.
  Phase A (steps 0..19 = i-chunks 0..3): one step per bank round-robin, so
    plane production (ACT tanh + fp8 copies, DVE power chain) stays ahead.
  Phase B (per bank, steps 20..39): back-to-back finish, staggered bank
    completion; evac + store overlap the next bank's matmuls.  Final group
    is evacuated in two column halves with the stores issued from the Sync
    and Scalar queues in parallel.
"""

import numpy as np
import ml_dtypes

B, I, O, D1 = 4096, 1024, 1024, 8
NCORES = 8
BS = B // NCORES       # batch rows per core (moving free dim of each matmul)
IC = I // 128          # i chunks
OC = O // 128          # o chunks (output partition tiles / PSUM banks)
SPI = 5                # steps per i-chunk: [pair(k1,k2), k3, k4, k5, pair(k6,k7)]
NJ_S = IC * SPI        # 40 steps per output group
NJA_S = 20             # phase-A steps (covers i-chunks 0..3)
STEP_B = 256           # weight-stream bytes per partition per step
NSTEP = OC * NJ_S      # 320 total steps
WSCALE = 8192.0        # 2^13 weight pre-scale (fp8 dynamic range)
FP8_MAX = 240.0        # TRN e4m3 saturates at +-240 (not OCP's 448)

# (oc, j) consumption order of the 320 weight-stream steps
SEQ = [(oc, j) for j in range(NJA_S) for oc in range(OC)] + \
      [(oc, j) for oc in range(OC) for j in range(NJA_S, NJ_S)]
# weight-DMA chunk sizes (steps): phase A starts fine-grained (the first
# chunk gates the first matmul) then coarsens; phase B is one chunk per
# output group
_SIZES = [4, 4, 8, 8, 8, 16, 16, 24, 24, 24, 24] + [NJ_S - NJA_S] * OC
CHUNKS = []
_s = 0
for _sz in _SIZES:
    CHUNKS.append((_s, _sz))
    _s += _sz
assert _s == NSTEP
_NA = len(_SIZES) - OC                       # number of phase-A chunks
GROUP_END_CHUNK = [_NA + oc for oc in range(OC)]

_GRAPH = None
LAST_RESULT = None     # BassKernelResults of the most recent run (for test.py)

# weight-chunk SBUF ring slots
CW_BUFS = 6
# sync-queue DMA issues hoisted ahead of the framework entry barrier
# (xin0 + the first HOIST_DMAS-1 weight chunks)
HOIST_DMAS = 4


def _step_meta(j):
    """kind ('p12'|'bf'|'p67'), bf16 power k, act/dve sem thresholds."""
    ic, s = divmod(j, SPI)
    if s == 0:
        return ic, 'p12', None, 3 * ic + 3, 0
    if s == 4:
        return ic, 'p67', None, 0, 6 * ic + 6
    return ic, 'bf', s + 2, 0, 6 * ic + s + 1   # s=1,2,3 -> k=3,4,5


def _build_graph_raw():
    import concourse.bass as bass
    from concourse import bacc, mybir

    nc = bacc.Bacc("TRN2", target_bir_lowering=False, debug=False,
                   num_devices=NCORES, monotonic_sem_count=0)
    f32 = mybir.dt.float32
    bf16 = mybir.dt.bfloat16
    fp8 = mybir.dt.float8e4

    xT = nc.dram_tensor("xT", [I, BS], bf16, kind="ExternalInput").ap()
    cw = nc.dram_tensor("cw", [128, NSTEP * STEP_B], fp8,
                        kind="ExternalInput").ap()
    bias = nc.dram_tensor("bias", [128, OC], f32, kind="ExternalInput").ap()
    yT = nc.dram_tensor("yT", [O, BS], f32, kind="ExternalOutput").ap()

    max_chunk = max(sz for _, sz in CHUNKS)
    xin = [nc.alloc_sbuf_tensor(f"xin{i}", [128, BS], bf16).ap()
           for i in range(IC)]
    # bf16 planes per i-chunk: index by k (1..5); k=1 is tanh
    pbf = [{k: nc.alloc_sbuf_tensor(f"pb{i}_{k}", [128, BS], bf16).ap()
            for k in range(1, 6)} for i in range(IC)]
    pr12 = [nc.alloc_sbuf_tensor(f"p12_{i}", [128, 2, BS], fp8).ap()
            for i in range(IC)]
    pr67 = [nc.alloc_sbuf_tensor(f"p67_{i}", [128, 2, BS], fp8).ap()
            for i in range(IC)]
    cwbuf = [nc.alloc_sbuf_tensor(f"cwb{i}", [128, max_chunk * STEP_B],
                                  fp8).ap()
             for i in range(CW_BUFS)]
    # never written: garbage operand for PE p-state warm-up matmuls
    warm2 = nc.alloc_sbuf_tensor("warm2", [128, BS], bf16).ap()
    bias_t = nc.alloc_sbuf_tensor("biasb", [128, OC], f32).ap()
    ot = [nc.alloc_sbuf_tensor(f"ot{i}", [128, BS], f32).ap()
          for i in range(2)]
    ps = [nc.alloc_psum_tensor(f"ps{i}", [128, BS], f32).ap()
          for i in range(OC)]
    HB = BS // 2

    from contextlib import ExitStack
    with ExitStack() as stack:
        # gpsimd issues only early DMAs whose completions are consumed mid-
        # kernel, so its expensive end-of-block dge_drain can be skipped
        block = stack.enter_context(nc.Block(no_gpsimd_drain=True))
        # DMA completion increments land as 16 per-slice +1s, and slices of
        # different in-flight DMAs interleave -- a semaphore may only be
        # waited at "all DMAs issued on it so far" thresholds.
        cw_dma = [stack.enter_context(nc.semaphore(f"cw_dma{r}"))
                  for r in range(CW_BUFS)]
        xin0_dma = stack.enter_context(nc.semaphore("xin0_dma"))
        # x tiles 1..7 ride gpsimd SWDGE with per-tile sems (SWDGE and
        # HWDGE DMAs may not mix on a sem)
        xr_dma = [stack.enter_context(nc.semaphore(f"xr_dma{i}"))
                  for i in range(IC - 1)]
        bias_dma = stack.enter_context(nc.semaphore("bias_dma"))
        out_dma = [stack.enter_context(nc.semaphore(f"out_dma{r}"))
                   for r in range(2)]
        act_pl = stack.enter_context(nc.semaphore("act_pl"))
        dve_pl = stack.enter_context(nc.semaphore("dve_pl"))
        pe_ch = stack.enter_context(nc.semaphore("pe_ch"))
        act_ev = stack.enter_context(nc.semaphore("act_ev"))
        dve_ev = stack.enter_context(nc.semaphore("dve_ev"))

        @block.sync
        def _(eng: bass.BassEngine):
            # xin0 first: it gates the whole plane pipeline.  This DMA and
            # the first weight chunks are hoisted pre-barrier below.
            eng.dma_start(out=xin[0][:], in_=xT[0:128, :]
                          ).then_inc(xin0_dma, 16)
            for ci, (s0, size) in enumerate(CHUNKS):
                if ci >= CW_BUFS:
                    eng.wait_ge(pe_ch, ci - CW_BUFS + 1)
                eng.dma_start(
                    out=cwbuf[ci % CW_BUFS][:, :size * STEP_B],
                    in_=cw[:, s0 * STEP_B:(s0 + size) * STEP_B],
                ).then_inc(cw_dma[ci % CW_BUFS], 16)
            # output stores: evac->store handoff runs here so the DMA issue
            # cost never serializes with the next evac on the Scalar queue
            for oc in range(OC - 1):
                eng.wait_ge(act_ev, oc + 1)
                eng.dma_start(
                    out=yT[oc * 128:(oc + 1) * 128, :],
                    in_=ot[oc % 2][:]
                ).then_inc(out_dma[oc % 2], 16)
            o0 = (OC - 1) * 128
            eng.wait_ge(act_ev, OC)
            eng.dma_start(out=yT[o0:o0 + 128, 0:HB], in_=ot[1][:, 0:HB]
                          ).then_inc(out_dma[1], 16)

        @block.gpsimd
        def _(eng: bass.BassEngine):
            # x tiles 1..7 + bias on the otherwise-idle SWDGE queue
            for i in range(1, IC):
                eng.dma_start(out=xin[i][:], in_=xT[i * 128:(i + 1) * 128, :]
                              ).then_inc(xr_dma[i - 1], 16)
            eng.dma_start(out=bias_t[:], in_=bias[:]).then_inc(bias_dma, 16)

        @block.scalar
        def _(eng: bass.BassEngine):
            # plane production: tanh (bf16 chain input) + the two fp8
            # copies of the (k1,k2) DoubleRow pair.  act_pl: 3 per i-chunk.
            for i in range(IC):
                if i == 0:
                    eng.wait_ge(xin0_dma, 16)
                else:
                    eng.wait_ge(xr_dma[i - 1], 16)
                eng.activation(pbf[i][1][:], xin[i][:],
                               mybir.ActivationFunctionType.Tanh
                               ).then_inc(act_pl, 1)
                eng.activation(pr12[i][:, 0], pbf[i][1][:],
                               mybir.ActivationFunctionType.Copy
                               ).then_inc(act_pl, 1)
                eng.wait_ge(dve_pl, 6 * i + 1)
                eng.activation(pr12[i][:, 1], pbf[i][2][:],
                               mybir.ActivationFunctionType.Copy
                               ).then_inc(act_pl, 1)
            eng.wait_ge(bias_dma, 16)
            for oc in range(OC - 1):
                eng.wait_ge(pe_ch, GROUP_END_CHUNK[oc] + 1)
                if oc >= 2:
                    eng.wait_ge(out_dma[oc % 2], 16 * (oc // 2))
                eng.activation(ot[oc % 2][:], ps[oc][:],
                               mybir.ActivationFunctionType.Identity,
                               bias=bias_t[:, oc:oc + 1],
                               scale=1.0 / WSCALE).then_inc(act_ev, 1)
            # last group: two half-column evacs; half A stores from Sync,
            # half B from here (Sync is busy issuing half A then)
            eng.wait_ge(pe_ch, len(CHUNKS))
            eng.wait_ge(out_dma[1], 16 * ((OC - 1) // 2))
            eng.activation(ot[1][:, 0:HB], ps[OC - 1][:, 0:HB],
                           mybir.ActivationFunctionType.Identity,
                           bias=bias_t[:, OC - 1:OC],
                           scale=1.0 / WSCALE).then_inc(act_ev, 1)
            eng.activation(ot[1][:, HB:BS], ps[OC - 1][:, HB:BS],
                           mybir.ActivationFunctionType.Identity,
                           bias=bias_t[:, OC - 1:OC],
                           scale=1.0 / WSCALE).then_inc(dve_ev, 1)
            o0 = (OC - 1) * 128
            eng.wait_ge(dve_ev, 1)
            eng.dma_start(out=yT[o0:o0 + 128, HB:BS],
                          in_=ot[1][:, HB:BS]).then_inc(out_dma[1], 16)
            # no final out-DMA waits: the runtime drains the queues

        @block.vector
        def _(eng: bass.BassEngine):
            # power chain t^2..t^5 in bf16 + the (k6,k7) fp8 pair.
            # dve_pl: 6 per i-chunk.  Same-engine RAW still needs a sem wait
            # (deep pipeline, no interlock).
            for i in range(IC):
                t = pbf[i][1]
                eng.wait_ge(act_pl, 3 * i + 1)
                eng.tensor_mul(pbf[i][2][:], t[:], t[:]).then_inc(dve_pl, 1)
                eng.wait_ge(dve_pl, 6 * i + 1)
                eng.tensor_mul(pbf[i][3][:], pbf[i][2][:], t[:]
                               ).then_inc(dve_pl, 1)
                eng.wait_ge(dve_pl, 6 * i + 2)
                eng.tensor_mul(pbf[i][4][:], pbf[i][3][:], t[:]
                               ).then_inc(dve_pl, 1)
                eng.wait_ge(dve_pl, 6 * i + 3)
                eng.tensor_mul(pbf[i][5][:], pbf[i][4][:], t[:]
                               ).then_inc(dve_pl, 1)
                eng.wait_ge(dve_pl, 6 * i + 4)
                eng.tensor_mul(pr67[i][:, 0], pbf[i][5][:], t[:]
                               ).then_inc(dve_pl, 1)
                eng.tensor_mul(pr67[i][:, 1], pbf[i][5][:], pbf[i][2][:]
                               ).then_inc(dve_pl, 1)

        @block.tensor
        def _(eng: bass.BassEngine):
            # p-state warm-up on garbage inputs while the first x tile +
            # weight chunk DMAs land
            for _ in range(9):
                eng.matmul(ps[0][:], warm2[:, 0:128], warm2[:],
                           start=True, stop=True)
            done = [0] * OC
            seen_act = seen_dve = 0
            sem_uses = [0] * CW_BUFS
            for ci, (s0, size) in enumerate(CHUNKS):
                js = [SEQ[s][1] for s in range(s0, s0 + size)]
                need_act = max(_step_meta(j)[3] for j in js)
                need_dve = max(_step_meta(j)[4] for j in js)
                if need_act > seen_act:
                    eng.wait_ge(act_pl, need_act)
                    seen_act = need_act
                if need_dve > seen_dve:
                    eng.wait_ge(dve_pl, need_dve)
                    seen_dve = need_dve
                for t in range(size):
                    oc, j = SEQ[s0 + t]
                    ic, kind, kk, _, _ = _step_meta(j)
                    sl = cwbuf[ci % CW_BUFS][:,
                                             t * STEP_B:(t + 1) * STEP_B]
                    if kind == 'bf':
                        mm = eng.matmul(ps[oc][:], sl.bitcast(bf16),
                                        pbf[ic][kk][:],
                                        start=(done[oc] == 0),
                                        stop=(done[oc] == NJ_S - 1))
                    else:
                        pair = pr12[ic] if kind == 'p12' else pr67[ic]
                        mm = eng.matmul(
                            ps[oc][:],
                            sl.rearrange("p (two f) -> p two f", two=2),
                            pair[:],
                            start=(done[oc] == 0),
                            stop=(done[oc] == NJ_S - 1),
                            perf_mode=mybir.MatmulPerfMode.DoubleRow)
                    done[oc] += 1
                    if t == 0:
                        sem_uses[ci % CW_BUFS] += 1
                        mm._wait_ge(cw_dma[ci % CW_BUFS],
                                    16 * sem_uses[ci % CW_BUFS])
                    if t == size - 1:
                        mm.then_inc(pe_ch, 1)
            assert all(d == NJ_S for d in done)

    # Hoist the first few Sync-queue DMA issues (xin0 + leading weight
    # chunks) into the entry block, ahead of the framework's all-engine
    # barrier: their ~0.6us-per-DMA descriptor generation then overlaps the
    # fixed ~7us NEFF preamble.  Safe: these DMAs write SBUF regions nothing
    # reads until their semaphores fire, and sems start at zero.
    from concourse import mybir as _mybir
    entry = nc.main_func.blocks[0]
    sp_eng = _mybir.EngineType.SP
    sp_body = next(
        b for b in nc.main_func.blocks
        if b.instructions and type(b.instructions[0]).__name__ == "InstDMACopy"
        and b.instructions[0].engine == sp_eng)
    moved = []
    for inst in list(sp_body.instructions):
        if len(moved) >= HOIST_DMAS:
            break
        if type(inst).__name__ != "InstDMACopy":
            break
        moved.append(inst)
    # place them AFTER SP's barrier-arrival drain (so the other engines'
    # release isn't delayed by the DMA issues) but BEFORE its release-wait
    bar_idx = next(
        i for i, inst in enumerate(entry.instructions)
        if type(inst).__name__ == "InstDrain" and inst.engine == sp_eng)
    for inst in moved:
        sp_body.instructions.remove(inst)
    for k, inst in enumerate(moved):
        entry.instructions.insert(bar_idx + 1 + k, inst)

    nc.compile()
    return nc


def _get_graph():
    global _GRAPH
    if _GRAPH is None:
        _GRAPH = _build_graph_raw()
    return _GRAPH


def _host_prep(a, q, coeffs):
    """Fold the polynomial basis change into the weights (float64 on host)
    and pack the mixed bf16/fp8 weight stream."""
    f8 = ml_dtypes.float8_e4m3fn
    bf = ml_dtypes.bfloat16
    # c[d, k]: P_d(t) = sum_k c[d, k] * t^k, from the three-term recurrence
    c = np.zeros((D1, D1), np.float64)
    c[0, 0] = 1.0
    if D1 > 1:
        c[1, 1] = 1.0
        c[1, 0] = -a
    for n in range(2, D1):
        c[n, 1:] += c[n - 1, :-1]
        c[n, :] -= (a + q ** n) * c[n - 1, :]
        c[n, :] -= a * q ** (n - 1) * c[n - 2, :]

    Cf = (coeffs.reshape(-1, D1).astype(np.float64) @ c).reshape(I, O, D1)
    bias_dev = np.ascontiguousarray(
        Cf[:, :, 0].sum(axis=0).astype(np.float32).reshape(OC, 128).T)

    W = Cf[:, :, 1:] * WSCALE                     # [I, O, 7], k index 0..6
    # per-(ic, oc) 128x128 tiles, k = 1..7
    Wt = W.reshape(IC, 128, OC, 128, 7)           # [ic, p, oc, ol, k-1]

    def tile(ic, oc, k):
        return Wt[ic, :, oc, :, k - 1]            # [128, 128] float64

    stream = np.zeros((128, NSTEP * STEP_B), np.uint8)
    for n, (oc, j) in enumerate(SEQ):
        ic, s = divmod(j, SPI)
        dst = stream[:, n * STEP_B:(n + 1) * STEP_B]
        if s in (1, 2, 3):
            tb = tile(ic, oc, s + 2).astype(np.float32).astype(bf)
            dst[:] = tb.view(np.uint8).reshape(128, STEP_B)
        else:
            ka, kb = (1, 2) if s == 0 else (6, 7)
            pa = np.clip(tile(ic, oc, ka), -FP8_MAX, FP8_MAX
                         ).astype(np.float32).astype(f8)
            pb = np.clip(tile(ic, oc, kb), -FP8_MAX, FP8_MAX
                         ).astype(np.float32).astype(f8)
            dst[:, 0:128] = pa.view(np.uint8)
            dst[:, 128:256] = pb.view(np.uint8)
    cw_dev = stream.view(f8)
    return cw_dev, bias_dev


def _ensure_axon_hooks_importable():
    """run_bass_kernel_spmd imports antenv.axon_hooks when BASS_TRACE is
    set; some images lack that module.  Register a no-op fallback so a
    trace request degrades to a warning instead of an ImportError."""
    import sys
    import types
    if "antenv.axon_hooks" in sys.modules:
        return
    try:
        import antenv.axon_hooks  # noqa: F401
    except ImportError:
        mod = types.ModuleType("antenv.axon_hooks")
        state = {"hook": None}
        mod.set_axon_ntff_profile_hook = \
            lambda h: state.__setitem__("hook", h)
        mod.get_axon_ntff_profile_hook = lambda: state["hook"]
        sys.modules["antenv.axon_hooks"] = mod
        try:
            import antenv
            antenv.axon_hooks = mod
        except ImportError:
            pass


def kernel(x, a, q, coeffs):
    global LAST_RESULT
    _ensure_axon_hooks_importable()
    from concourse.bass_utils import run_bass_kernel_spmd

    x = np.ascontiguousarray(np.asarray(x, dtype=np.float32))
    coeffs = np.ascontiguousarray(np.asarray(coeffs, dtype=np.float32))
    a_val = float(np.asarray(a).reshape(-1)[0])
    q_val = float(np.asarray(q).reshape(-1)[0])

    cw_dev, bias_dev = _host_prep(a_val, q_val, coeffs)
    xs = x.reshape(NCORES, BS, I).transpose(0, 2, 1)  # [core, I, BS]
    xs = xs.astype(ml_dtypes.bfloat16)

    in_maps = [{
        "xT": np.ascontiguousarray(xs[c]),
        "cw": cw_dev,
        "bias": bias_dev,
    } for c in range(NCORES)]

    nc = _get_graph()
    res = run_bass_kernel_spmd(nc, in_maps, core_ids=list(range(NCORES)))
    LAST_RESULT = res

    shards = [np.asarray(res.results[c]["yT"]).T for c in range(NCORES)]
    return np.ascontiguousarray(np.concatenate(shards, axis=0),
                                dtype=np.float32)


if __name__ == "__main__":
    rng = np.random.default_rng(0)
    inputs = {
        "x": rng.standard_normal((B, I), dtype=np.float32),
        "a": np.zeros((1,), np.float32),
        "q": np.ones((1,), np.float32),
        "coeffs": rng.standard_normal((I, O, D1), dtype=np.float32)
        / (I * D1),
    }
    y = kernel(**inputs)
    print("out", y.shape, y.dtype, float(np.abs(y).mean()))


# revision 61
# speedup vs baseline: 1.4203x; 1.0830x over previous
"""Al-Salam-Carlitz KAN layer on 8 TRN2 NeuronCores.

Math: y[b,o] = sum_{i,d} P_d(tanh(x[b,i])) * coeffs[i,o,d], where P_d are the
Al-Salam-Carlitz polynomials given by a three-term recurrence in scalars a, q.
Each P_d is a degree-d polynomial in t = tanh(x), so on the host we fold the
(D+1)x(D+1) basis-change matrix into coeffs:

    y[b,o] = bias[o] + sum_{k=1..7} sum_i t[b,i]^k * Cf[i,o,k]

with bias[o] = sum_i Cf[i,o,0] (the k=0 column times t^0 == 1).

Mixed precision: after basis folding the per-k weight norms are wildly
uneven -- k=3,4,5 carry ~89% of the output variance, k=1,2,6,7 only ~11%.
The low-variance planes run as fp8-e4m3 DoubleRow matmuls (2 K-tiles per
instruction, measured 2x bf16 throughput at 512 moving cols), the heavy
k=3,4,5 stay bf16.  Per output group: 8 i-chunks x (1 pair(k1,k2) +
3 bf16 singles + 1 pair(k6,k7)) = 40 matmul steps instead of 56.
Measured end-to-end rel err ~1e-2 vs the 2e-2 gate.

fp8 weight encoding needs a scale: the folded weights (sigma ~1e-4..2e-3)
sit below e4m3's subnormal floor, so ALL weights are pre-scaled by 2^13 on
the host and the PSUM is descaled in the evacuation (activation
out = in*scale + bias with scale = 2^-13, an exact power of two).

Sharding: data-parallel over batch (4096 -> 8 x 512).  Each core receives
its x-shard pre-transposed ([I, 512] bf16), the folded weight stream (one
fp8-typed byte stream; bf16 tiles are bitcast views, every step is 256
bytes/partition), and the bias.  No collectives.

Schedule (one core): 8 PSUM banks, each accumulating its 40 steps.
  Entry: the first 4 Sync DMA issues (xin0 + 3 weight chunks) are hoisted
    into the NEFF entry block ahead of the framework's all-engine barrier,
    so their descriptors generate during the fixed ~7us preamble.
  Warm-up: 5 dummy matmuls on garbage ramp the PE p-state while the first
    tanh/fp8-pair is still in flight.
  Phase A (steps 0..19 = i-chunks 0..3): one step per bank round-robin, so
    plane production (ACT tanh + fp8 copies, DVE power chain) stays ahead.
  Phase B (per bank, steps 20..39): back-to-back finish, staggered bank
    completion; evac + store overlap the next bank's matmuls.  Final group
    is evacuated in two column halves with the stores issued from the Sync
    and Scalar queues in parallel.
"""

import numpy as np
import ml_dtypes

B, I, O, D1 = 4096, 1024, 1024, 8
NCORES = 8
BS = B // NCORES       # batch rows per core (moving free dim of each matmul)
IC = I // 128          # i chunks
OC = O // 128          # o chunks (output partition tiles / PSUM banks)
STEP_B = 256           # weight-stream bytes per partition per step
WSCALE = 8192.0        # 2^13 weight pre-scale (fp8 dynamic range)
FP8_MAX = 240.0        # TRN e4m3 saturates at +-240 (not OCP's 448)

# Step table per output group.  fp8 planes: k=1,2 pair per i-chunk, k=6,7
# pair per i-chunk, and k=5 paired ACROSS adjacent i-chunks (a DoubleRow
# pair may contract any two K-tiles).  bf16 singles: k=3,4.
# 9 steps per i-chunk pair -> 36 per group (vs 56 all-bf16 K-steps).
STEPS = []
for _icp in range(IC // 2):
    _a, _b = 2 * _icp, 2 * _icp + 1
    STEPS += [('p12', _a), ('bf3', _a), ('bf4', _a),
              ('p12', _b), ('bf3', _b), ('bf4', _b),
              ('p55', _icp), ('p67', _a), ('p67', _b)]
NJ_S = len(STEPS)      # 36 steps per output group
NJA_S = 18             # phase-A steps (covers i-chunks 0..3)
NSTEP = OC * NJ_S      # 288 total steps

# PE semaphore thresholds per step.  act_pl: 3/i-chunk (tanh, p12a,
# t5fp8-copy); dve_pl: 7/i-chunk (t2, p12b, t3, t4, t5, p67a, p67b).
def _step_need(st):
    kind = st[0]
    if kind == 'p12':
        return 3 * st[1] + 2, 7 * st[1] + 2
    if kind == 'bf3':
        return 0, 7 * st[1] + 3
    if kind == 'bf4':
        return 0, 7 * st[1] + 4
    if kind == 'p55':
        return 3 * (2 * st[1] + 1) + 3, 0
    return 0, 7 * st[1] + 7          # p67


# (oc, j) consumption order of the weight-stream steps
SEQ = [(oc, j) for j in range(NJA_S) for oc in range(OC)] + \
      [(oc, j) for oc in range(OC) for j in range(NJA_S, NJ_S)]
# weight-DMA chunk sizes (steps): phase A starts fine-grained (the first
# chunk gates the first matmul) then coarsens; phase B is one chunk per
# output group
_SIZES = [4, 4, 4, 8, 8, 8, 12, 16, 16, 16, 24, 24] + [NJ_S - NJA_S] * OC
CHUNKS = []
_s = 0
for _sz in _SIZES:
    CHUNKS.append((_s, _sz))
    _s += _sz
assert _s == NSTEP
_NA = len(_SIZES) - OC                       # number of phase-A chunks
GROUP_END_CHUNK = [_NA + oc for oc in range(OC)]

_GRAPH = None
LAST_RESULT = None     # BassKernelResults of the most recent run (for test.py)

# weight-chunk SBUF ring slots
CW_BUFS = 6
# sync-queue DMA issues hoisted ahead of the framework entry barrier
# (xin0 + the first HOIST_DMAS-1 weight chunks)
HOIST_DMAS = 4


def _build_graph_raw():
    import concourse.bass as bass
    from concourse import bacc, mybir

    nc = bacc.Bacc("TRN2", target_bir_lowering=False, debug=False,
                   num_devices=NCORES, monotonic_sem_count=0)
    f32 = mybir.dt.float32
    bf16 = mybir.dt.bfloat16
    fp8 = mybir.dt.float8e4

    xT = nc.dram_tensor("xT", [I, BS], bf16, kind="ExternalInput").ap()
    cw = nc.dram_tensor("cw", [128, NSTEP * STEP_B], fp8,
                        kind="ExternalInput").ap()
    bias = nc.dram_tensor("bias", [128, OC], f32, kind="ExternalInput").ap()
    yT = nc.dram_tensor("yT", [O, BS], f32, kind="ExternalOutput").ap()

    max_chunk = max(sz for _, sz in CHUNKS)
    xin = [nc.alloc_sbuf_tensor(f"xin{i}", [128, BS], bf16).ap()
           for i in range(IC)]
    # bf16 planes per i-chunk: index by k (1..5); k=1 is tanh
    pbf = [{k: nc.alloc_sbuf_tensor(f"pb{i}_{k}", [128, BS], bf16).ap()
            for k in range(1, 6)} for i in range(IC)]
    pr12 = [nc.alloc_sbuf_tensor(f"p12_{i}", [128, 2, BS], fp8).ap()
            for i in range(IC)]
    pr67 = [nc.alloc_sbuf_tensor(f"p67_{i}", [128, 2, BS], fp8).ap()
            for i in range(IC)]
    pr55 = [nc.alloc_sbuf_tensor(f"p55_{i}", [128, 2, BS], fp8).ap()
            for i in range(IC // 2)]
    cwbuf = [nc.alloc_sbuf_tensor(f"cwb{i}", [128, max_chunk * STEP_B],
                                  fp8).ap()
             for i in range(CW_BUFS)]
    # never written: garbage operand for PE p-state warm-up matmuls
    warm2 = nc.alloc_sbuf_tensor("warm2", [128, BS], bf16).ap()
    bias_t = nc.alloc_sbuf_tensor("biasb", [128, OC], f32).ap()
    ot = [nc.alloc_sbuf_tensor(f"ot{i}", [128, BS], f32).ap()
          for i in range(2)]
    ps = [nc.alloc_psum_tensor(f"ps{i}", [128, BS], f32).ap()
          for i in range(OC)]
    HB = BS // 2

    from contextlib import ExitStack
    with ExitStack() as stack:
        # gpsimd issues only early DMAs whose completions are consumed mid-
        # kernel, so its expensive end-of-block dge_drain can be skipped
        block = stack.enter_context(nc.Block(no_gpsimd_drain=True))
        # DMA completion increments land as 16 per-slice +1s, and slices of
        # different in-flight DMAs interleave -- a semaphore may only be
        # waited at "all DMAs issued on it so far" thresholds.
        cw_dma = [stack.enter_context(nc.semaphore(f"cw_dma{r}"))
                  for r in range(CW_BUFS)]
        xin0_dma = stack.enter_context(nc.semaphore("xin0_dma"))
        # x tiles 1..7 ride gpsimd SWDGE with per-tile sems (SWDGE and
        # HWDGE DMAs may not mix on a sem)
        xr_dma = [stack.enter_context(nc.semaphore(f"xr_dma{i}"))
                  for i in range(IC - 1)]
        bias_dma = stack.enter_context(nc.semaphore("bias_dma"))
        out_dma = [stack.enter_context(nc.semaphore(f"out_dma{r}"))
                   for r in range(2)]
        act_pl = stack.enter_context(nc.semaphore("act_pl"))
        dve_pl = stack.enter_context(nc.semaphore("dve_pl"))
        pe_ch = stack.enter_context(nc.semaphore("pe_ch"))
        act_ev = stack.enter_context(nc.semaphore("act_ev"))
        dve_ev = stack.enter_context(nc.semaphore("dve_ev"))

        @block.sync
        def _(eng: bass.BassEngine):
            # xin0 first: it gates the whole plane pipeline.  This DMA and
            # the first weight chunks are hoisted pre-barrier below.
            eng.dma_start(out=xin[0][:], in_=xT[0:128, :]
                          ).then_inc(xin0_dma, 16)
            for ci, (s0, size) in enumerate(CHUNKS):
                if ci >= CW_BUFS:
                    eng.wait_ge(pe_ch, ci - CW_BUFS + 1)
                eng.dma_start(
                    out=cwbuf[ci % CW_BUFS][:, :size * STEP_B],
                    in_=cw[:, s0 * STEP_B:(s0 + size) * STEP_B],
                ).then_inc(cw_dma[ci % CW_BUFS], 16)
            # output stores: evac->store handoff runs here so the DMA issue
            # cost never serializes with the next evac on the Scalar queue
            for oc in range(OC - 1):
                eng.wait_ge(act_ev, oc + 1)
                eng.dma_start(
                    out=yT[oc * 128:(oc + 1) * 128, :],
                    in_=ot[oc % 2][:]
                ).then_inc(out_dma[oc % 2], 16)
            o0 = (OC - 1) * 128
            eng.wait_ge(act_ev, OC)
            eng.dma_start(out=yT[o0:o0 + 128, 0:HB], in_=ot[1][:, 0:HB]
                          ).then_inc(out_dma[1], 16)

        @block.gpsimd
        def _(eng: bass.BassEngine):
            # x tiles 1..7 + bias on the otherwise-idle SWDGE queue
            for i in range(1, IC):
                eng.dma_start(out=xin[i][:], in_=xT[i * 128:(i + 1) * 128, :]
                              ).then_inc(xr_dma[i - 1], 16)
            eng.dma_start(out=bias_t[:], in_=bias[:]).then_inc(bias_dma, 16)

        @block.scalar
        def _(eng: bass.BassEngine):
            # plane production: tanh (bf16 chain input), fp8 copy of t (the
            # k1 pair half), fp8 copy of t^5 into the cross-i-chunk k5
            # pair.  act_pl: 3 per i-chunk.
            for i in range(IC):
                if i == 0:
                    eng.wait_ge(xin0_dma, 16)
                else:
                    eng.wait_ge(xr_dma[i - 1], 16)
                eng.activation(pbf[i][1][:], xin[i][:],
                               mybir.ActivationFunctionType.Tanh
                               ).then_inc(act_pl, 1)
                eng.activation(pr12[i][:, 0], pbf[i][1][:],
                               mybir.ActivationFunctionType.Copy
                               ).then_inc(act_pl, 1)
                eng.wait_ge(dve_pl, 7 * i + 5)
                eng.activation(pr55[i // 2][:, i % 2], pbf[i][5][:],
                               mybir.ActivationFunctionType.Copy
                               ).then_inc(act_pl, 1)
            eng.wait_ge(bias_dma, 16)
            for oc in range(OC - 1):
                eng.wait_ge(pe_ch, GROUP_END_CHUNK[oc] + 1)
                if oc >= 2:
                    eng.wait_ge(out_dma[oc % 2], 16 * (oc // 2))
                eng.activation(ot[oc % 2][:], ps[oc][:],
                               mybir.ActivationFunctionType.Identity,
                               bias=bias_t[:, oc:oc + 1],
                               scale=1.0 / WSCALE).then_inc(act_ev, 1)
            # last group: two half-column evacs; half A stores from Sync,
            # half B from here (Sync is busy issuing half A then)
            eng.wait_ge(pe_ch, len(CHUNKS))
            eng.wait_ge(out_dma[1], 16 * ((OC - 1) // 2))
            eng.activation(ot[1][:, 0:HB], ps[OC - 1][:, 0:HB],
                           mybir.ActivationFunctionType.Identity,
                           bias=bias_t[:, OC - 1:OC],
                           scale=1.0 / WSCALE).then_inc(act_ev, 1)
            eng.activation(ot[1][:, HB:BS], ps[OC - 1][:, HB:BS],
                           mybir.ActivationFunctionType.Identity,
                           bias=bias_t[:, OC - 1:OC],
                           scale=1.0 / WSCALE).then_inc(dve_ev, 1)
            o0 = (OC - 1) * 128
            eng.wait_ge(dve_ev, 1)
            eng.dma_start(out=yT[o0:o0 + 128, HB:BS],
                          in_=ot[1][:, HB:BS]).then_inc(out_dma[1], 16)
            # no final out-DMA waits: the runtime drains the queues

        @block.vector
        def _(eng: bass.BassEngine):
            # power chain t^2..t^5 in bf16, the fp8 t^2 (k2 pair half) and
            # the (k6,k7) fp8 pair.  dve_pl: 7 per i-chunk.  Same-engine
            # RAW still needs a sem wait (deep pipeline, no interlock).
            for i in range(IC):
                t = pbf[i][1]
                eng.wait_ge(act_pl, 3 * i + 1)
                eng.tensor_mul(pbf[i][2][:], t[:], t[:]).then_inc(dve_pl, 1)
                eng.tensor_mul(pr12[i][:, 1], t[:], t[:]).then_inc(dve_pl, 1)
                eng.wait_ge(dve_pl, 7 * i + 1)
                eng.tensor_mul(pbf[i][3][:], pbf[i][2][:], t[:]
                               ).then_inc(dve_pl, 1)
                eng.wait_ge(dve_pl, 7 * i + 3)
                eng.tensor_mul(pbf[i][4][:], pbf[i][3][:], t[:]
                               ).then_inc(dve_pl, 1)
                eng.wait_ge(dve_pl, 7 * i + 4)
                eng.tensor_mul(pbf[i][5][:], pbf[i][4][:], t[:]
                               ).then_inc(dve_pl, 1)
                eng.wait_ge(dve_pl, 7 * i + 5)
                eng.tensor_mul(pr67[i][:, 0], pbf[i][5][:], t[:]
                               ).then_inc(dve_pl, 1)
                eng.tensor_mul(pr67[i][:, 1], pbf[i][5][:], pbf[i][2][:]
                               ).then_inc(dve_pl, 1)

        @block.tensor
        def _(eng: bass.BassEngine):
            # p-state warm-up on garbage inputs while the first x tile +
            # weight chunk DMAs land
            for _ in range(9):
                eng.matmul(ps[0][:], warm2[:, 0:128], warm2[:],
                           start=True, stop=True)
            done = [0] * OC
            seen_act = seen_dve = 0
            sem_uses = [0] * CW_BUFS
            for ci, (s0, size) in enumerate(CHUNKS):
                needs = [_step_need(STEPS[SEQ[s][1]])
                         for s in range(s0, s0 + size)]
                need_act = max(n[0] for n in needs)
                need_dve = max(n[1] for n in needs)
                if need_act > seen_act:
                    eng.wait_ge(act_pl, need_act)
                    seen_act = need_act
                if need_dve > seen_dve:
                    eng.wait_ge(dve_pl, need_dve)
                    seen_dve = need_dve
                for t in range(size):
                    oc, j = SEQ[s0 + t]
                    kind, idx = STEPS[j]
                    sl = cwbuf[ci % CW_BUFS][:,
                                             t * STEP_B:(t + 1) * STEP_B]
                    if kind in ('bf3', 'bf4'):
                        mm = eng.matmul(ps[oc][:], sl.bitcast(bf16),
                                        pbf[idx][3 if kind == 'bf3'
                                                 else 4][:],
                                        start=(done[oc] == 0),
                                        stop=(done[oc] == NJ_S - 1))
                    else:
                        pair = {'p12': pr12, 'p67': pr67,
                                'p55': pr55}[kind][idx]
                        mm = eng.matmul(
                            ps[oc][:],
                            sl.rearrange("p (two f) -> p two f", two=2),
                            pair[:],
                            start=(done[oc] == 0),
                            stop=(done[oc] == NJ_S - 1),
                            perf_mode=mybir.MatmulPerfMode.DoubleRow)
                    done[oc] += 1
                    if t == 0:
                        sem_uses[ci % CW_BUFS] += 1
                        mm._wait_ge(cw_dma[ci % CW_BUFS],
                                    16 * sem_uses[ci % CW_BUFS])
                    if t == size - 1:
                        mm.then_inc(pe_ch, 1)
            assert all(d == NJ_S for d in done)

    # Hoist the first few Sync-queue DMA issues (xin0 + leading weight
    # chunks) into the entry block, ahead of the framework's all-engine
    # barrier: their ~0.6us-per-DMA descriptor generation then overlaps the
    # fixed ~7us NEFF preamble.  Safe: these DMAs write SBUF regions nothing
    # reads until their semaphores fire, and sems start at zero.
    from concourse import mybir as _mybir
    entry = nc.main_func.blocks[0]
    sp_eng = _mybir.EngineType.SP
    sp_body = next(
        b for b in nc.main_func.blocks
        if b.instructions and type(b.instructions[0]).__name__ == "InstDMACopy"
        and b.instructions[0].engine == sp_eng)
    moved = []
    for inst in list(sp_body.instructions):
        if len(moved) >= HOIST_DMAS:
            break
        if type(inst).__name__ != "InstDMACopy":
            break
        moved.append(inst)
    # place them AFTER SP's barrier-arrival drain (so the other engines'
    # release isn't delayed by the DMA issues) but BEFORE its release-wait
    bar_idx = next(
        i for i, inst in enumerate(entry.instructions)
        if type(inst).__name__ == "InstDrain" and inst.engine == sp_eng)
    for inst in moved:
        sp_body.instructions.remove(inst)
    for k, inst in enumerate(moved):
        entry.instructions.insert(bar_idx + 1 + k, inst)

    nc.compile()
    return nc


def _get_graph():
    global _GRAPH
    if _GRAPH is None:
        _GRAPH = _build_graph_raw()
    return _GRAPH


def _host_prep(a, q, coeffs):
    """Fold the polynomial basis change into the weights (float64 on host)
    and pack the mixed bf16/fp8 weight stream."""
    f8 = ml_dtypes.float8_e4m3fn
    bf = ml_dtypes.bfloat16
    # c[d, k]: P_d(t) = sum_k c[d, k] * t^k, from the three-term recurrence
    c = np.zeros((D1, D1), np.float64)
    c[0, 0] = 1.0
    if D1 > 1:
        c[1, 1] = 1.0
        c[1, 0] = -a
    for n in range(2, D1):
        c[n, 1:] += c[n - 1, :-1]
        c[n, :] -= (a + q ** n) * c[n - 1, :]
        c[n, :] -= a * q ** (n - 1) * c[n - 2, :]

    Cf = (coeffs.reshape(-1, D1).astype(np.float64) @ c).reshape(I, O, D1)
    bias_dev = np.ascontiguousarray(
        Cf[:, :, 0].sum(axis=0).astype(np.float32).reshape(OC, 128).T)

    W = Cf[:, :, 1:] * WSCALE                     # [I, O, 7], k index 0..6
    # per-(ic, oc) 128x128 tiles, k = 1..7
    Wt = W.reshape(IC, 128, OC, 128, 7)           # [ic, p, oc, ol, k-1]

    def tile(ic, oc, k):
        return Wt[ic, :, oc, :, k - 1]            # [128, 128] float64

    def f8tile(ic, oc, k):
        return np.clip(tile(ic, oc, k), -FP8_MAX, FP8_MAX
                       ).astype(np.float32).astype(f8)

    stream = np.zeros((128, NSTEP * STEP_B), np.uint8)
    for n, (oc, j) in enumerate(SEQ):
        kind, idx = STEPS[j]
        dst = stream[:, n * STEP_B:(n + 1) * STEP_B]
        if kind == 'bf3' or kind == 'bf4':
            k = 3 if kind == 'bf3' else 4
            tb = tile(idx, oc, k).astype(np.float32).astype(bf)
            dst[:] = tb.view(np.uint8).reshape(128, STEP_B)
        else:
            if kind == 'p12':
                pa, pb = f8tile(idx, oc, 1), f8tile(idx, oc, 2)
            elif kind == 'p67':
                pa, pb = f8tile(idx, oc, 6), f8tile(idx, oc, 7)
            else:   # p55: k=5 of adjacent i-chunks
                pa = f8tile(2 * idx, oc, 5)
                pb = f8tile(2 * idx + 1, oc, 5)
            dst[:, 0:128] = pa.view(np.uint8)
            dst[:, 128:256] = pb.view(np.uint8)
    cw_dev = stream.view(f8)
    return cw_dev, bias_dev


def _ensure_axon_hooks_importable():
    """run_bass_kernel_spmd imports antenv.axon_hooks when BASS_TRACE is
    set; some images lack that module.  Register a no-op fallback so a
    trace request degrades to a warning instead of an ImportError."""
    import sys
    import types
    if "antenv.axon_hooks" in sys.modules:
        return
    try:
        import antenv.axon_hooks  # noqa: F401
    except ImportError:
        mod = types.ModuleType("antenv.axon_hooks")
        state = {"hook": None}
        mod.set_axon_ntff_profile_hook = \
            lambda h: state.__setitem__("hook", h)
        mod.get_axon_ntff_profile_hook = lambda: state["hook"]
        sys.modules["antenv.axon_hooks"] = mod
        try:
            import antenv
            antenv.axon_hooks = mod
        except ImportError:
            pass


def kernel(x, a, q, coeffs):
    global LAST_RESULT
    _ensure_axon_hooks_importable()
    from concourse.bass_utils import run_bass_kernel_spmd

    x = np.ascontiguousarray(np.asarray(x, dtype=np.float32))
    coeffs = np.ascontiguousarray(np.asarray(coeffs, dtype=np.float32))
    a_val = float(np.asarray(a).reshape(-1)[0])
    q_val = float(np.asarray(q).reshape(-1)[0])

    cw_dev, bias_dev = _host_prep(a_val, q_val, coeffs)
    xs = x.reshape(NCORES, BS, I).transpose(0, 2, 1)  # [core, I, BS]
    xs = xs.astype(ml_dtypes.bfloat16)

    in_maps = [{
        "xT": np.ascontiguousarray(xs[c]),
        "cw": cw_dev,
        "bias": bias_dev,
    } for c in range(NCORES)]

    nc = _get_graph()
    res = run_bass_kernel_spmd(nc, in_maps, core_ids=list(range(NCORES)))
    LAST_RESULT = res

    shards = [np.asarray(res.results[c]["yT"]).T for c in range(NCORES)]
    return np.ascontiguousarray(np.concatenate(shards, axis=0),
                                dtype=np.float32)


if __name__ == "__main__":
    rng = np.random.default_rng(0)
    inputs = {
        "x": rng.standard_normal((B, I), dtype=np.float32),
        "a": np.zeros((1,), np.float32),
        "q": np.ones((1,), np.float32),
        "coeffs": rng.standard_normal((I, O, D1), dtype=np.float32)
        / (I * D1),
    }
    y = kernel(**inputs)
    print("out", y.shape, y.dtype, float(np.abs(y).mean()))


# revision 67
# speedup vs baseline: 1.4398x; 1.0137x over previous
"""Al-Salam-Carlitz KAN layer on 8 TRN2 NeuronCores.

Math: y[b,o] = sum_{i,d} P_d(tanh(x[b,i])) * coeffs[i,o,d], where P_d are the
Al-Salam-Carlitz polynomials given by a three-term recurrence in scalars a, q.
Each P_d is a degree-d polynomial in t = tanh(x), so on the host we fold the
(D+1)x(D+1) basis-change matrix into coeffs:

    y[b,o] = bias[o] + sum_{k=1..7} sum_i t[b,i]^k * Cf[i,o,k]

with bias[o] = sum_i Cf[i,o,0] (the k=0 column times t^0 == 1).

Mixed precision: after basis folding the per-k weight norms are wildly
uneven -- k=3,4,5 carry ~89% of the output variance, k=1,2,6,7 only ~11%.
The low-variance planes run as fp8-e4m3 DoubleRow matmuls (2 K-tiles per
instruction, measured 2x bf16 throughput at 512 moving cols), the heavy
k=3,4,5 stay bf16.  Per output group: 8 i-chunks x (1 pair(k1,k2) +
3 bf16 singles + 1 pair(k6,k7)) = 40 matmul steps instead of 56.
Measured end-to-end rel err ~1e-2 vs the 2e-2 gate.

fp8 weight encoding needs a scale: the folded weights (sigma ~1e-4..2e-3)
sit below e4m3's subnormal floor, so ALL weights are pre-scaled by 2^13 on
the host and the PSUM is descaled in the evacuation (activation
out = in*scale + bias with scale = 2^-13, an exact power of two).

Sharding: data-parallel over batch (4096 -> 8 x 512).  Each core receives
its x-shard pre-transposed ([I, 512] bf16), the folded weight stream (one
fp8-typed byte stream; bf16 tiles are bitcast views, every step is 256
bytes/partition), and the bias.  No collectives.

Schedule (one core): 8 PSUM banks, each accumulating its 40 steps.
  Entry: the first 4 Sync DMA issues (xin0 + 3 weight chunks) are hoisted
    into the NEFF entry block ahead of the framework's all-engine barrier,
    so their descriptors generate during the fixed ~7us preamble.
  Warm-up: 5 dummy matmuls on garbage ramp the PE p-state while the first
    tanh/fp8-pair is still in flight.
  Phase A (steps 0..19 = i-chunks 0..3): one step per bank round-robin, so
    plane production (ACT tanh + fp8 copies, DVE power chain) stays ahead.
  Phase B (per bank, steps 20..39): back-to-back finish, staggered bank
    completion; evac + store overlap the next bank's matmuls.  Final group
    is evacuated in two column halves with the stores issued from the Sync
    and Scalar queues in parallel.
"""

import numpy as np
import ml_dtypes

B, I, O, D1 = 4096, 1024, 1024, 8
NCORES = 8
BS = B // NCORES       # batch rows per core (moving free dim of each matmul)
IC = I // 128          # i chunks
OC = O // 128          # o chunks (output partition tiles / PSUM banks)
STEP_B = 256           # weight-stream bytes per partition per step
WSCALE = 8192.0        # 2^13 weight pre-scale (fp8 dynamic range)
FP8_MAX = 240.0        # TRN e4m3 saturates at +-240 (not OCP's 448)

# Step table per output group.  fp8 planes: k=1,2 pair per i-chunk, k=6,7
# pair per i-chunk, and k=5 paired ACROSS adjacent i-chunks (a DoubleRow
# pair may contract any two K-tiles).  bf16 singles: k=3,4.
# 9 steps per i-chunk pair -> 36 per group (vs 56 all-bf16 K-steps).
STEPS = []
for _icp in range(IC // 2):
    _a, _b = 2 * _icp, 2 * _icp + 1
    STEPS += [('p12', _a), ('bf3', _a), ('bf4', _a),
              ('p12', _b), ('bf3', _b), ('bf4', _b),
              ('p55', _icp), ('p67', _a), ('p67', _b)]
NJ_S = len(STEPS)      # 36 steps per output group
NJA_S = 18             # phase-A steps (covers i-chunks 0..3)
NSTEP = OC * NJ_S      # 288 total steps

# PE semaphore thresholds per step.  act_pl: 3/i-chunk (tanh, p12a,
# t5fp8-copy); dve_pl: 7/i-chunk (t2, p12b, t3, t4, t5, p67a, p67b).
def _step_need(st):
    kind = st[0]
    if kind == 'p12':
        return 3 * st[1] + 2, 7 * st[1] + 1
    if kind == 'bf3':
        return 0, 7 * st[1] + 3
    if kind == 'bf4':
        return 0, 7 * st[1] + 4
    if kind == 'p55':
        return 3 * (2 * st[1] + 1) + 3, 0
    return 0, 7 * st[1] + 7          # p67


# (oc, j) consumption order of the weight-stream steps
SEQ = [(oc, j) for j in range(NJA_S) for oc in range(OC)] + \
      [(oc, j) for oc in range(OC) for j in range(NJA_S, NJ_S)]
# weight-DMA chunk sizes (steps): phase A starts fine-grained (the first
# chunk gates the first matmul) then coarsens; phase B is one chunk per
# output group
_SIZES = [4, 4, 4, 8, 8, 8, 12, 16, 16, 16, 24, 24] + [NJ_S - NJA_S] * OC
CHUNKS = []
_s = 0
for _sz in _SIZES:
    CHUNKS.append((_s, _sz))
    _s += _sz
assert _s == NSTEP
_NA = len(_SIZES) - OC                       # number of phase-A chunks
GROUP_END_CHUNK = [_NA + oc for oc in range(OC)]

_GRAPH = None
LAST_RESULT = None     # BassKernelResults of the most recent run (for test.py)

# weight-chunk SBUF ring slots
CW_BUFS = 6
# sync-queue DMA issues hoisted ahead of the framework entry barrier
# (xin0 + the first HOIST_DMAS-1 weight chunks)
HOIST_DMAS = 4


def _build_graph_raw():
    import concourse.bass as bass
    from concourse import bacc, mybir

    nc = bacc.Bacc("TRN2", target_bir_lowering=False, debug=False,
                   num_devices=NCORES, monotonic_sem_count=0)
    f32 = mybir.dt.float32
    bf16 = mybir.dt.bfloat16
    fp8 = mybir.dt.float8e4

    xT = nc.dram_tensor("xT", [I, BS], bf16, kind="ExternalInput").ap()
    cw = nc.dram_tensor("cw", [128, NSTEP * STEP_B], fp8,
                        kind="ExternalInput").ap()
    bias = nc.dram_tensor("bias", [128, OC], f32, kind="ExternalInput").ap()
    yT = nc.dram_tensor("yT", [O, BS], f32, kind="ExternalOutput").ap()

    max_chunk = max(sz for _, sz in CHUNKS)
    xin = [nc.alloc_sbuf_tensor(f"xin{i}", [128, BS], bf16).ap()
           for i in range(IC)]
    # bf16 planes per i-chunk: index by k (1..5); k=1 is tanh
    pbf = [{k: nc.alloc_sbuf_tensor(f"pb{i}_{k}", [128, BS], bf16).ap()
            for k in range(1, 6)} for i in range(IC)]
    pr12 = [nc.alloc_sbuf_tensor(f"p12_{i}", [128, 2, BS], fp8).ap()
            for i in range(IC)]
    pr67 = [nc.alloc_sbuf_tensor(f"p67_{i}", [128, 2, BS], fp8).ap()
            for i in range(IC)]
    pr55 = [nc.alloc_sbuf_tensor(f"p55_{i}", [128, 2, BS], fp8).ap()
            for i in range(IC // 2)]
    cwbuf = [nc.alloc_sbuf_tensor(f"cwb{i}", [128, max_chunk * STEP_B],
                                  fp8).ap()
             for i in range(CW_BUFS)]
    # never written: garbage operand for PE p-state warm-up matmuls
    warm2 = nc.alloc_sbuf_tensor("warm2", [128, BS], bf16).ap()
    bias_t = nc.alloc_sbuf_tensor("biasb", [128, OC], f32).ap()
    ot = [nc.alloc_sbuf_tensor(f"ot{i}", [128, BS], f32).ap()
          for i in range(2)]
    ps = [nc.alloc_psum_tensor(f"ps{i}", [128, BS], f32).ap()
          for i in range(OC)]
    HB = BS // 2

    from contextlib import ExitStack
    with ExitStack() as stack:
        # gpsimd issues only early DMAs whose completions are consumed mid-
        # kernel, so its expensive end-of-block dge_drain can be skipped
        block = stack.enter_context(nc.Block(no_gpsimd_drain=True))
        # DMA completion increments land as 16 per-slice +1s, and slices of
        # different in-flight DMAs interleave -- a semaphore may only be
        # waited at "all DMAs issued on it so far" thresholds.
        cw_dma = [stack.enter_context(nc.semaphore(f"cw_dma{r}"))
                  for r in range(CW_BUFS)]
        xin0_dma = stack.enter_context(nc.semaphore("xin0_dma"))
        # x tiles 1..7 ride gpsimd SWDGE with per-tile sems (SWDGE and
        # HWDGE DMAs may not mix on a sem)
        xr_dma = [stack.enter_context(nc.semaphore(f"xr_dma{i}"))
                  for i in range(IC - 1)]
        bias_dma = stack.enter_context(nc.semaphore("bias_dma"))
        cwg = stack.enter_context(nc.semaphore("cwg"))
        out_dma = [stack.enter_context(nc.semaphore(f"out_dma{r}"))
                   for r in range(2)]
        act_pl = stack.enter_context(nc.semaphore("act_pl"))
        dve_pl = stack.enter_context(nc.semaphore("dve_pl"))
        pe_ch = stack.enter_context(nc.semaphore("pe_ch"))
        act_ev = stack.enter_context(nc.semaphore("act_ev"))
        dve_ev = stack.enter_context(nc.semaphore("dve_ev"))

        @block.sync
        def _(eng: bass.BassEngine):
            # xin0 first: it gates the whole plane pipeline.  This DMA and
            # the first weight chunks are hoisted pre-barrier below.
            eng.dma_start(out=xin[0][:], in_=xT[0:128, :]
                          ).then_inc(xin0_dma, 16)
            for ci, (s0, size) in enumerate(CHUNKS):
                if ci == 2:
                    continue     # chunk 2 rides the gpsimd SWDGE queue
                if ci >= CW_BUFS:
                    eng.wait_ge(pe_ch, ci - CW_BUFS + 1)
                eng.dma_start(
                    out=cwbuf[ci % CW_BUFS][:, :size * STEP_B],
                    in_=cw[:, s0 * STEP_B:(s0 + size) * STEP_B],
                ).then_inc(cw_dma[ci % CW_BUFS], 16)
            # output stores: evac->store handoff runs here so the DMA issue
            # cost never serializes with the next evac on the Scalar queue
            for oc in range(OC - 1):
                eng.wait_ge(act_ev, oc + 1)
                eng.dma_start(
                    out=yT[oc * 128:(oc + 1) * 128, :],
                    in_=ot[oc % 2][:]
                ).then_inc(out_dma[oc % 2], 16)
            o0 = (OC - 1) * 128
            eng.wait_ge(act_ev, OC)
            eng.dma_start(out=yT[o0:o0 + 128, 0:HB], in_=ot[1][:, 0:HB]
                          ).then_inc(out_dma[1], 16)

        @block.gpsimd
        def _(eng: bass.BassEngine):
            # weight chunk 2 + x tiles 1..7 + bias on the otherwise-idle
            # SWDGE queue (parallel channel to the Sync HWDGE early burst)
            s0, size = CHUNKS[2]
            eng.dma_start(
                out=cwbuf[2][:, :size * STEP_B],
                in_=cw[:, s0 * STEP_B:(s0 + size) * STEP_B],
            ).then_inc(cwg, 16)
            for i in range(1, IC):
                eng.dma_start(out=xin[i][:], in_=xT[i * 128:(i + 1) * 128, :]
                              ).then_inc(xr_dma[i - 1], 16)
            eng.dma_start(out=bias_t[:], in_=bias[:]).then_inc(bias_dma, 16)

        @block.scalar
        def _(eng: bass.BassEngine):
            # plane production: tanh (bf16 chain input), fp8 copy of t (the
            # k1 pair half), fp8 copy of t^5 into the cross-i-chunk k5
            # pair.  act_pl: 3 per i-chunk.
            for i in range(IC):
                if i == 0:
                    eng.wait_ge(xin0_dma, 16)
                else:
                    eng.wait_ge(xr_dma[i - 1], 16)
                eng.activation(pbf[i][1][:], xin[i][:],
                               mybir.ActivationFunctionType.Tanh
                               ).then_inc(act_pl, 1)
                eng.activation(pr12[i][:, 0], pbf[i][1][:],
                               mybir.ActivationFunctionType.Copy
                               ).then_inc(act_pl, 1)
                eng.wait_ge(dve_pl, 7 * i + 5)
                eng.activation(pr55[i // 2][:, i % 2], pbf[i][5][:],
                               mybir.ActivationFunctionType.Copy
                               ).then_inc(act_pl, 1)
            eng.wait_ge(bias_dma, 16)
            for oc in range(OC - 1):
                eng.wait_ge(pe_ch, GROUP_END_CHUNK[oc] + 1)
                if oc >= 2:
                    eng.wait_ge(out_dma[oc % 2], 16 * (oc // 2))
                eng.activation(ot[oc % 2][:], ps[oc][:],
                               mybir.ActivationFunctionType.Identity,
                               bias=bias_t[:, oc:oc + 1],
                               scale=1.0 / WSCALE).then_inc(act_ev, 1)
            # last group: two half-column evacs; half A stores from Sync,
            # half B from here (Sync is busy issuing half A then)
            eng.wait_ge(pe_ch, len(CHUNKS))
            eng.wait_ge(out_dma[1], 16 * ((OC - 1) // 2))
            eng.activation(ot[1][:, 0:HB], ps[OC - 1][:, 0:HB],
                           mybir.ActivationFunctionType.Identity,
                           bias=bias_t[:, OC - 1:OC],
                           scale=1.0 / WSCALE).then_inc(act_ev, 1)
            eng.activation(ot[1][:, HB:BS], ps[OC - 1][:, HB:BS],
                           mybir.ActivationFunctionType.Identity,
                           bias=bias_t[:, OC - 1:OC],
                           scale=1.0 / WSCALE).then_inc(dve_ev, 1)
            o0 = (OC - 1) * 128
            eng.wait_ge(dve_ev, 1)
            eng.dma_start(out=yT[o0:o0 + 128, HB:BS],
                          in_=ot[1][:, HB:BS]).then_inc(out_dma[1], 16)
            # no final out-DMA waits: the runtime drains the queues

        @block.vector
        def _(eng: bass.BassEngine):
            # power chain t^2..t^5 in bf16, the fp8 t^2 (k2 pair half) and
            # the (k6,k7) fp8 pair.  dve_pl: 7 per i-chunk.  Same-engine
            # RAW still needs a sem wait (deep pipeline, no interlock).
            for i in range(IC):
                t = pbf[i][1]
                eng.wait_ge(act_pl, 3 * i + 1)
                eng.tensor_mul(pr12[i][:, 1], t[:], t[:]).then_inc(dve_pl, 1)
                eng.tensor_mul(pbf[i][2][:], t[:], t[:]).then_inc(dve_pl, 1)
                eng.wait_ge(dve_pl, 7 * i + 2)
                eng.tensor_mul(pbf[i][3][:], pbf[i][2][:], t[:]
                               ).then_inc(dve_pl, 1)
                eng.wait_ge(dve_pl, 7 * i + 3)
                eng.tensor_mul(pbf[i][4][:], pbf[i][3][:], t[:]
                               ).then_inc(dve_pl, 1)
                eng.wait_ge(dve_pl, 7 * i + 4)
                eng.tensor_mul(pbf[i][5][:], pbf[i][4][:], t[:]
                               ).then_inc(dve_pl, 1)
                eng.wait_ge(dve_pl, 7 * i + 5)
                eng.tensor_mul(pr67[i][:, 0], pbf[i][5][:], t[:]
                               ).then_inc(dve_pl, 1)
                eng.tensor_mul(pr67[i][:, 1], pbf[i][5][:], pbf[i][2][:]
                               ).then_inc(dve_pl, 1)

        @block.tensor
        def _(eng: bass.BassEngine):
            # p-state warm-up on garbage inputs while the first x tile +
            # weight chunk DMAs land
            for _ in range(9):
                eng.matmul(ps[0][:], warm2[:, 0:128], warm2[:],
                           start=True, stop=True)
            done = [0] * OC
            seen_act = seen_dve = 0
            sem_uses = [0] * CW_BUFS
            for ci, (s0, size) in enumerate(CHUNKS):
                needs = [_step_need(STEPS[SEQ[s][1]])
                         for s in range(s0, s0 + size)]
                need_act = max(n[0] for n in needs)
                need_dve = max(n[1] for n in needs)
                if need_act > seen_act:
                    eng.wait_ge(act_pl, need_act)
                    seen_act = need_act
                if need_dve > seen_dve:
                    eng.wait_ge(dve_pl, need_dve)
                    seen_dve = need_dve
                for t in range(size):
                    oc, j = SEQ[s0 + t]
                    kind, idx = STEPS[j]
                    sl = cwbuf[ci % CW_BUFS][:,
                                             t * STEP_B:(t + 1) * STEP_B]
                    if kind in ('bf3', 'bf4'):
                        mm = eng.matmul(ps[oc][:], sl.bitcast(bf16),
                                        pbf[idx][3 if kind == 'bf3'
                                                 else 4][:],
                                        start=(done[oc] == 0),
                                        stop=(done[oc] == NJ_S - 1))
                    else:
                        pair = {'p12': pr12, 'p67': pr67,
                                'p55': pr55}[kind][idx]
                        mm = eng.matmul(
                            ps[oc][:],
                            sl.rearrange("p (two f) -> p two f", two=2),
                            pair[:],
                            start=(done[oc] == 0),
                            stop=(done[oc] == NJ_S - 1),
                            perf_mode=mybir.MatmulPerfMode.DoubleRow)
                    done[oc] += 1
                    if t == 0:
                        if ci == 2:
                            mm._wait_ge(cwg, 16)
                        else:
                            sem_uses[ci % CW_BUFS] += 1
                            mm._wait_ge(cw_dma[ci % CW_BUFS],
                                        16 * sem_uses[ci % CW_BUFS])
                    if t == size - 1:
                        mm.then_inc(pe_ch, 1)
            assert all(d == NJ_S for d in done)

    # Hoist the first few Sync-queue DMA issues (xin0 + leading weight
    # chunks) into the entry block, ahead of the framework's all-engine
    # barrier: their ~0.6us-per-DMA descriptor generation then overlaps the
    # fixed ~7us NEFF preamble.  Safe: these DMAs write SBUF regions nothing
    # reads until their semaphores fire, and sems start at zero.
    from concourse import mybir as _mybir
    entry = nc.main_func.blocks[0]
    sp_eng = _mybir.EngineType.SP
    sp_body = next(
        b for b in nc.main_func.blocks
        if b.instructions and type(b.instructions[0]).__name__ == "InstDMACopy"
        and b.instructions[0].engine == sp_eng)
    moved = []
    for inst in list(sp_body.instructions):
        if len(moved) >= HOIST_DMAS:
            break
        if type(inst).__name__ != "InstDMACopy":
            break
        moved.append(inst)
    # place them AFTER SP's barrier-arrival drain (so the other engines'
    # release isn't delayed by the DMA issues) but BEFORE its release-wait
    bar_idx = next(
        i for i, inst in enumerate(entry.instructions)
        if type(inst).__name__ == "InstDrain" and inst.engine == sp_eng)
    for inst in moved:
        sp_body.instructions.remove(inst)
    for k, inst in enumerate(moved):
        entry.instructions.insert(bar_idx + 1 + k, inst)

    nc.compile()
    return nc


def _get_graph():
    global _GRAPH
    if _GRAPH is None:
        _GRAPH = _build_graph_raw()
    return _GRAPH


def _host_prep(a, q, coeffs):
    """Fold the polynomial basis change into the weights (float64 on host)
    and pack the mixed bf16/fp8 weight stream."""
    f8 = ml_dtypes.float8_e4m3fn
    bf = ml_dtypes.bfloat16
    # c[d, k]: P_d(t) = sum_k c[d, k] * t^k, from the three-term recurrence
    c = np.zeros((D1, D1), np.float64)
    c[0, 0] = 1.0
    if D1 > 1:
        c[1, 1] = 1.0
        c[1, 0] = -a
    for n in range(2, D1):
        c[n, 1:] += c[n - 1, :-1]
        c[n, :] -= (a + q ** n) * c[n - 1, :]
        c[n, :] -= a * q ** (n - 1) * c[n - 2, :]

    Cf = (coeffs.reshape(-1, D1).astype(np.float64) @ c).reshape(I, O, D1)
    bias_dev = np.ascontiguousarray(
        Cf[:, :, 0].sum(axis=0).astype(np.float32).reshape(OC, 128).T)

    W = Cf[:, :, 1:] * WSCALE                     # [I, O, 7], k index 0..6
    # per-(ic, oc) 128x128 tiles, k = 1..7
    Wt = W.reshape(IC, 128, OC, 128, 7)           # [ic, p, oc, ol, k-1]

    def tile(ic, oc, k):
        return Wt[ic, :, oc, :, k - 1]            # [128, 128] float64

    def f8tile(ic, oc, k):
        return np.clip(tile(ic, oc, k), -FP8_MAX, FP8_MAX
                       ).astype(np.float32).astype(f8)

    stream = np.zeros((128, NSTEP * STEP_B), np.uint8)
    for n, (oc, j) in enumerate(SEQ):
        kind, idx = STEPS[j]
        dst = stream[:, n * STEP_B:(n + 1) * STEP_B]
        if kind == 'bf3' or kind == 'bf4':
            k = 3 if kind == 'bf3' else 4
            tb = tile(idx, oc, k).astype(np.float32).astype(bf)
            dst[:] = tb.view(np.uint8).reshape(128, STEP_B)
        else:
            if kind == 'p12':
                pa, pb = f8tile(idx, oc, 1), f8tile(idx, oc, 2)
            elif kind == 'p67':
                pa, pb = f8tile(idx, oc, 6), f8tile(idx, oc, 7)
            else:   # p55: k=5 of adjacent i-chunks
                pa = f8tile(2 * idx, oc, 5)
                pb = f8tile(2 * idx + 1, oc, 5)
            dst[:, 0:128] = pa.view(np.uint8)
            dst[:, 128:256] = pb.view(np.uint8)
    cw_dev = stream.view(f8)
    return cw_dev, bias_dev


def _ensure_axon_hooks_importable():
    """run_bass_kernel_spmd imports antenv.axon_hooks when BASS_TRACE is
    set; some images lack that module.  Register a no-op fallback so a
    trace request degrades to a warning instead of an ImportError."""
    import sys
    import types
    if "antenv.axon_hooks" in sys.modules:
        return
    try:
        import antenv.axon_hooks  # noqa: F401
    except ImportError:
        mod = types.ModuleType("antenv.axon_hooks")
        state = {"hook": None}
        mod.set_axon_ntff_profile_hook = \
            lambda h: state.__setitem__("hook", h)
        mod.get_axon_ntff_profile_hook = lambda: state["hook"]
        sys.modules["antenv.axon_hooks"] = mod
        try:
            import antenv
            antenv.axon_hooks = mod
        except ImportError:
            pass


def kernel(x, a, q, coeffs):
    global LAST_RESULT
    _ensure_axon_hooks_importable()
    from concourse.bass_utils import run_bass_kernel_spmd

    x = np.ascontiguousarray(np.asarray(x, dtype=np.float32))
    coeffs = np.ascontiguousarray(np.asarray(coeffs, dtype=np.float32))
    a_val = float(np.asarray(a).reshape(-1)[0])
    q_val = float(np.asarray(q).reshape(-1)[0])

    cw_dev, bias_dev = _host_prep(a_val, q_val, coeffs)
    xs = x.reshape(NCORES, BS, I).transpose(0, 2, 1)  # [core, I, BS]
    xs = xs.astype(ml_dtypes.bfloat16)

    in_maps = [{
        "xT": np.ascontiguousarray(xs[c]),
        "cw": cw_dev,
        "bias": bias_dev,
    } for c in range(NCORES)]

    nc = _get_graph()
    res = run_bass_kernel_spmd(nc, in_maps, core_ids=list(range(NCORES)))
    LAST_RESULT = res

    shards = [np.asarray(res.results[c]["yT"]).T for c in range(NCORES)]
    return np.ascontiguousarray(np.concatenate(shards, axis=0),
                                dtype=np.float32)


if __name__ == "__main__":
    rng = np.random.default_rng(0)
    inputs = {
        "x": rng.standard_normal((B, I), dtype=np.float32),
        "a": np.zeros((1,), np.float32),
        "q": np.ones((1,), np.float32),
        "coeffs": rng.standard_normal((I, O, D1), dtype=np.float32)
        / (I * D1),
    }
    y = kernel(**inputs)
    print("out", y.shape, y.dtype, float(np.abs(y).mean()))
